# revision 47
# baseline (speedup 1.0000x reference)
"""CapsNet (nn_CapsNetBasic) forward pass as a Bass/Tile kernel on 8 TRN2 cores.

Sharding: 8 cores = 2 batch samples x 4 row-blocks of 32 output rows each.
Every core computes its 32x128-pixel slab end-to-end:
  conv1 (5x5, 1->256, bf16 im2col matmul with fused valid-mask/bias rows)
  primary caps conv (5x5, 256->256) in fp8e4m3 via DoubleRow matmuls: one
    instruction contracts both 128-channel halves per tap (25 matmuls/chain
    instead of 50) -- activations scaled x8, weights x64, rescaled in the
    preact activation (1/16384).
  per-capsule squash (partition-group reductions via 0/1 indicator matmuls)
  seg caps accumulated across 4 row-blocks into one PSUM tile at partition
    offsets {0,32,64,96} so the scalar tail (seg squash, length, masking,
    sigmoid) runs once per superblock on batched [4,512]/[128,512] tiles.
  recon 1x1 convs (16->64->128->1) per block, sigmoid via odd polynomial.
Superblock-0's tail matmuls are interleaved between superblock-1's primary
chains to keep the PE queue streaming; the final tail runs in two column
halves to shorten the drain. Routing softmaxes are constant for these shapes
(uniform 1/32 and singleton 1.0), so routing reduces to fixed reductions.

Scheduling notes (HW-measured):
- All tensor-path data is bf16/fp8: fp32 rhs runs the PE at 1/4 rate.
- The HAM clock gate starts the PE at 1.2 GHz and drops back whenever the
  PE idles ~a 3.4us window; a full-array dummy-matmul accumulation chain
  pre-warms it during the input DMA ramp, and short dummy chains woven
  into the final tail keep it warm across the serial squash spine.
- Matmuls emitted back-to-back whose lhsT tiles occupy disjoint PE row
  bands (tile_position) execute concurrently: used for conv1 quarter
  pairs, the bc broadcast m0/m1 pair, and the recon r1/r2 K=64 pairs.
- Engine queues are strictly in-order, so emission order must match
  data-ready order per engine; conv1's later-quarter evacuations go
  scalar-only so the vector queue never blocks a squash chain.
"""

import sys

sys.path.insert(0, "/opt/trn_rl_repo")

import numpy as np
import ml_dtypes
from contextlib import ExitStack

import concourse.bass as bass
import concourse.tile as tile
from concourse import mybir, bacc
from concourse.bass_utils import run_bass_kernel_spmd

F32 = mybir.dt.float32
F32R = mybir.dt.float32r
BF16 = mybir.dt.bfloat16
F8 = mybir.dt.float8e4
AF = mybir.ActivationFunctionType
DRMODE = mybir.MatmulPerfMode.DoubleRow
ADD = mybir.AluOpType.add
MULT = mybir.AluOpType.mult

B = 2
H = W = 128
RB = 32          # output rows per core
NBLK = 4         # row blocks per sample
NCORES = 8
RR = RB + 4      # conv1 buffer rows (halo 2 each side)
CW = W + 4       # padded width
AFLAT = RR * CW  # 4752
NPX = RB * W     # 4096 output pixels per core
QW = AFLAT // 4  # 1188 = 9 rows per conv1 quarter

SA = 8.0         # conv1-activation fp8 scale
SW = 64.0        # primary-conv weight fp8 scale
PSCALE = 1.0 / (32.0 * SA * SW)   # preact = psum*PSCALE + cb1

NP_BF16 = ml_dtypes.bfloat16
NP_F8 = ml_dtypes.float8_e4m3

INPUT_SHAPES = {
    "A4": (128, QW),              # bf16 im2col quarters
    "W1T4": (128, 256),           # bf16 conv1 weights (x SA)
    "WT8": (128, 2, 25, 256),     # fp8 primary conv weights (x SW)
    "YV": (NPX,),                 # f32 labels
    "PACKB": (128, 1924),         # bf16 matmul-constant pack
    "PACKF": (128, 9),            # f32 bias/eps pack
}

_PROGRAM = None


def _build_program():
    nc = bacc.Bacc("TRN2", target_bir_lowering=False, debug=False, num_devices=NCORES)

    d = {}
    dts = {"A4": BF16, "W1T4": BF16, "WT8": F8, "YV": F32,
           "PACKB": BF16, "PACKF": F32}
    for name, shape in INPUT_SHAPES.items():
        d[name] = nc.dram_tensor(name, list(shape), dts[name], kind="ExternalInput").ap()
    for name in ("OSEG", "OREC"):
        d[name] = nc.dram_tensor(name, [NPX], BF16, kind="ExternalOutput").ap()

    with tile.TileContext(nc) as tc, ExitStack() as ctx:
        pers = ctx.enter_context(tc.tile_pool(name="pers", bufs=1))
        pa = ctx.enter_context(tc.tile_pool(name="act", bufs=3))
        pb = ctx.enter_context(tc.tile_pool(name="bft", bufs=4))
        pt = ctx.enter_context(tc.tile_pool(name="tsm", bufs=4))
        ppc = ctx.enter_context(tc.tile_pool(name="ppc", bufs=2, space="PSUM"))
        pps = ctx.enter_context(tc.tile_pool(name="pps", bufs=3, space="PSUM"))

        # ---- persistent loads, ordered by earliest consumer. sync queue:
        # conv1's W1T4+A4 then the m0 DR weights in dy chunks (the m0 chains
        # consume taps incrementally); scalar queue: biases, the squash/seg
        # consts (PACKB cols <900), the m1 DR weights, then the tail consts.
        W1T4 = pers.tile([128, 256], BF16, tag="W1T4")
        nc.sync.dma_start(W1T4[:], d["W1T4"][:])
        A4 = pers.tile([128, QW], BF16, tag="A4")
        nc.sync.dma_start(A4[0:64, :], d["A4"][0:64, :])
        nc.sync.dma_start(A4[64:128, :], d["A4"][64:128, :])
        WT8 = pers.tile([128, 2, 25, 256], F8, tag="WT8")
        for dy0, dy1 in ((0, 2), (2, 4), (4, 5)):
            nc.sync.dma_start(WT8[:, 0, 5 * dy0:5 * dy1, :],
                              d["WT8"][:, 0, 5 * dy0:5 * dy1, :])
        PACKF = pers.tile([128, 9], F32, tag="PACKF")
        nc.scalar.dma_start(PACKF[:], d["PACKF"][:])
        PACKB = pers.tile([128, 1924], BF16, tag="PACKB")
        nc.scalar.dma_start(PACKB[:, 0:900], d["PACKB"][:, 0:900])
        for dy0, dy1 in ((0, 2), (2, 4), (4, 5)):
            nc.scalar.dma_start(WT8[:, 1, 5 * dy0:5 * dy1, :],
                                d["WT8"][:, 1, 5 * dy0:5 * dy1, :])
        nc.scalar.dma_start(PACKB[:, 900:1924], d["PACKB"][:, 900:1924])

        # ---- PE pre-warm. The HAM clock gate holds the PE at 1.2 GHz until
        # it has been busy ~3.4us; dummy matmuls on a zeroed scratch warm it
        # while the input DMAs are in flight, and later keep it warm across
        # the serial tail so the real tail matmuls run at 2.4 GHz.
        pdum = ctx.enter_context(tc.tile_pool(name="dum", bufs=1, space="PSUM"))
        DSC = pers.tile([128, 640], BF16, tag="DSC")
        nc.gpsimd.memset(DSC[:], 0.0)

        def warm(n, wN=512):
            # one full-array accumulation chain -> no inter-matmul semaphores
            # and full PE activity (HAM's busy detector needs both)
            dps = pdum.tile([128, 512], F32, tag="dum", name="warm")
            for i in range(n):
                nc.tensor.matmul(dps[:, :wN], DSC[:, 0:128],
                                 DSC[:, 128:128 + wN],
                                 start=(i == 0), stop=(i == n - 1))

        warm(15)

        # const views (all matmul outputs land at partition 0 or use
        # 32-aligned row bands; cross-partition placement happens via
        # zero-padded lhsT columns + psum accumulation)
        IND2a = PACKB[0:16, 644:772]     # capsule->atom broadcast, m=0 rows
        IND2b = PACKB[32:48, 644:772]    # copy for m=1 rows
        BCIND = PACKB[0:4, 772:900]      # block->group broadcast
        W2V = [PACKB[0:64, 900:1028], PACKB[64:128, 1028:1156]]
        WR3T = [PACKB[:, 1156 + 128 * j:1284 + 128 * j] for j in range(4)]
        W1PAIR = [PACKB[0:64, 1668:1796], PACKB[64:128, 1796:1924]]
        INDSQ = [PACKB[:, 64 * m:64 * m + 64] for m in range(2)]
        INDSQ16 = [PACKB[:, 0:16], PACKB[:, 96:112]]
        WsT4 = [PACKB[:, 128 + 128 * j:256 + 128 * j] for j in range(4)]
        INDSEG = PACKB[0:112, 640:644]   # group->block 0/1 reduction
        CB1 = PACKF[:, 0:2]
        CB2V = PACKF[:, 2:3]
        EPS48 = PACKF[0:48, 3:4]
        EPS4 = PACKF[0:4, 4:5]
        BR1 = PACKF[:, 5:6]              # rows 64-127 zero
        BR2 = PACKF[:, 6:7]
        BR3V = PACKF[0:97, 7:8]          # br3 at rows {0,32,64,96}

        # ---- conv1: 1->256 5x5 via host im2col (25 taps + valid-mask + bias
        # rows), 4 column-quarters on PE row bands {0,32,64,96}. relu out in
        # fp8 (x SA folded into W1T4), split across ACT (m=0) and Pool (m=1).
        C1 = pers.tile([128, 2, RR, CW], F8, tag="C1")
        C1F = [C1[:, m, :, :].rearrange("p r c -> p (r c)") for m in range(2)]
        _c1ctr = [0]

        def conv1_pair(qa, qb, scalar_only=False):
            """One m-half, one qoff chunk at a time for quarters qa and qb:
            the two matmuls sit in disjoint PE row bands (32qa / 32qb) so
            they execute concurrently. scalar_only keeps the vector engine
            free for squash chains when a pair runs between DR chains."""
            for m in range(2):
                for qoff in range(0, QW, 512):
                    n = min(512, QW - qoff)
                    ps2 = [None, None]
                    for i, qt in enumerate((qa, qb)):
                        # scalar_only pairs run between DR chains: keep their
                        # psums out of ppc so chains never wait on them via
                        # pool rotation
                        pool = ppc if (i == 0 and not scalar_only) else pps
                        ps2[i] = pool.tile([128, 512], F32,
                                           tag="ppc" if pool is ppc else "pps",
                                           name="c1ps")
                        nc.tensor.matmul(
                            ps2[i][:, :n],
                            W1T4[32 * qt:32 * qt + 27, m * 128:(m + 1) * 128],
                            A4[32 * qt:32 * qt + 27, qoff:qoff + n],
                            start=True, stop=True,
                            tile_position=(32 * qt, 0),
                        )
                    for i, qt in enumerate((qa, qb)):
                        _c1ctr[0] += 1
                        dst = C1F[m][:, QW * qt + qoff:QW * qt + qoff + n]
                        if scalar_only or _c1ctr[0] % 2 == 0:
                            nc.scalar.activation(dst, ps2[i][:, :n], AF.Relu,
                                                 bias=0.0, scale=1.0)
                        else:
                            nc.vector.tensor_scalar_max(dst, ps2[i][:, :n],
                                                        0.0)

        N = 512

        psp = ctx.enter_context(tc.tile_pool(name="psp", bufs=1, space="PSUM"))

        bst = {}

        def block_front(row0):
            """Primary DR conv + capsule square/reduce + squash scalar chain
            for one 4-row block. Both m halves' |s|^2 land in one SQ64 psum
            tile (m0 at partitions 0-15, m1 at 32-47) via zero-padded
            accumulating indicator matmuls."""
            st = {}
            bst[row0] = st
            P = [None, None]
            for m in range(2):
                ps = ppc.tile([128, 512], F32, tag="ppc")
                for t in range(25):
                    dy, dx = divmod(t, 5)
                    nc.tensor.matmul(
                        ps[:, :N],
                        WT8[:, :, t, m * 128:(m + 1) * 128],
                        C1[:, :, row0 + dy:row0 + dy + 4, dx:dx + 128],
                        start=(t == 0), stop=(t == 24),
                        perf_mode=DRMODE,
                    )
                P[m] = pb.tile([128, 512], BF16, tag="P", name="P")
                nc.scalar.activation(P[m][:, :N], ps[:, :N], AF.Identity,
                                     bias=CB1[:, m:m + 1], scale=PSCALE)
            st["P"] = P
            sq = pps.tile([64, 512], F32, tag="pps", name="sq")
            for m in range(2):
                S = pb.tile([128, 512], BF16, tag="S", name="S")
                nc.vector.tensor_mul(out=S[:, :N], in0=P[m][:, :N],
                                     in1=P[m][:, :N])
                nc.tensor.matmul(sq[0:64, :N], INDSQ[m][:], S[:, :N],
                                 start=(m == 0), stop=(m == 1))
            tq = pt.tile([48, 512], F32, tag="tq")
            nc.scalar.activation(tq[:, :N], sq[0:48, :N], AF.Sqrt,
                                 bias=EPS48[:], scale=1.0)
            u = pt.tile([48, 512], F32, tag="u")
            nc.vector.scalar_tensor_tensor(
                out=u[:, :N], in0=sq[0:48, :N], scalar=1.0, in1=tq[:, :N],
                op0=ADD, op1=MULT)
            rf0 = pt.tile([48, 512], F32, tag="rf0")
            nc.vector.reciprocal_approx_fast(out=rf0[:, :N], in_=u[:, :N])
            rf = pt.tile([48, 512], BF16, tag="rf")
            st["rf"] = rf
            nc.vector.tensor_mul(out=rf[:, :N], in0=sq[0:48, :N],
                                 in1=rf0[:, :N])

        def block_last_m(row0, m):
            """One m-half of the last block: DR chain + squash chain, so
            ready-early matmuls can be emitted between the two halves."""
            if row0 not in bst:
                rfb = pt.tile([48, 512], BF16, tag="rf", name="rf")
                bst[row0] = {"P": [None, None],
                             "rfs": [rfb[0:16, :], rfb[32:48, :]]}
            st = bst[row0]
            ps = ppc.tile([128, 512], F32, tag="ppc", name="ps")
            for t in range(25):
                dy, dx = divmod(t, 5)
                nc.tensor.matmul(
                    ps[:, :N],
                    WT8[:, :, t, m * 128:(m + 1) * 128],
                    C1[:, :, row0 + dy:row0 + dy + 4, dx:dx + 128],
                    start=(t == 0), stop=(t == 24),
                    perf_mode=DRMODE,
                )
            st["P"][m] = P = pb.tile([128, 512], BF16, tag="P", name="P")
            nc.scalar.activation(P[:, :N], ps[:, :N], AF.Identity,
                                 bias=CB1[:, m:m + 1], scale=PSCALE)
            S = pb.tile([128, 512], BF16, tag="S", name="S")
            nc.vector.tensor_mul(out=S[:, :N], in0=P[:, :N], in1=P[:, :N])
            sq = pps.tile([16, 512], F32, tag="pps", name="sq")
            nc.tensor.matmul(sq[0:16, :N], INDSQ16[m][:], S[:, :N],
                             start=True, stop=True)
            tq = pt.tile([16, 512], F32, tag="tq")
            nc.scalar.activation(tq[:, :N], sq[0:16, :N], AF.Sqrt,
                                 bias=EPS48[0:16], scale=1.0)
            u = pt.tile([16, 512], F32, tag="u")
            nc.vector.scalar_tensor_tensor(
                out=u[:, :N], in0=sq[0:16, :N], scalar=1.0,
                in1=tq[:, :N], op0=ADD, op1=MULT)
            rf0 = pt.tile([16, 512], F32, tag="rf0")
            nc.vector.reciprocal_approx_fast(out=rf0[:, :N], in_=u[:, :N])
            nc.vector.tensor_mul(out=st["rfs"][m][:, :N], in0=sq[0:16, :N],
                                 in1=rf0[:, :N])

        def block_last_back(row0, sbst):
            st = bst.pop(row0)
            j = (row0 % 16) // 4
            spp = sbst["spp"]
            P, rfs = st["P"], st["rfs"]
            bc = [None, None]
            for m in range(2):
                bc[m] = pps.tile([128, 512], F32, tag="pps", name="bc")
                nc.tensor.matmul(bc[m][:, :N], IND2a[:] if m == 0 else IND2b[:],
                                 rfs[m][:, :N], start=True, stop=True,
                                 tile_position=(32 * m, 0))
            pm = [None, None]
            for m in range(2):
                pm[m] = pb.tile([128, 512], BF16, tag="pm", name="pm")
                nc.vector.tensor_mul(out=pm[m][:, :N], in0=P[m][:, :N],
                                     in1=bc[m][:, :N])
            for m in range(2):
                nc.tensor.matmul(spp[:, :N], WsT4[j][:], pm[m][:, :N],
                                 start=False, stop=(m == 1))

        def block_back(row0, sbst):
            """Broadcast squash factors, apply, seg conv accumulating into
            the superblock's SPP128 (block j at partitions 32j via
            zero-padded WsT columns). Emitted one block behind the fronts
            so the PE never waits on the squash chain."""
            st = bst.pop(row0)
            j = (row0 % 16) // 4
            if j == 0:
                sbst["spp"] = psp.tile([128, 512], F32, tag="spp", name="spp")
            spp = sbst["spp"]
            # bc m0/m1 sit in disjoint PE row bands (0-15 / 32-47): emitted
            # back-to-back they execute concurrently in the array.
            bc = [None, None]
            for m in range(2):
                bc[m] = pps.tile([128, 512], F32, tag="pps", name="bc")
                nc.tensor.matmul(bc[m][:, :N], IND2a[:] if m == 0 else IND2b[:],
                                 st["rf"][32 * m:32 * m + 16, :N],
                                 start=True, stop=True,
                                 tile_position=(32 * m, 0))
            pm = [None, None]
            for m in range(2):
                pm[m] = pb.tile([128, 512], BF16, tag="pm", name="pm")
                nc.vector.tensor_mul(out=pm[m][:, :N], in0=st["P"][m][:, :N],
                                     in1=bc[m][:, :N])
            for m in range(2):
                nc.tensor.matmul(spp[:, :N], WsT4[j][:], pm[m][:, :N],
                                 start=(j == 0 and m == 0),
                                 stop=(j == 3 and m == 1))

        # ---- superblock tail (pixel range [p0, p0+2048), blocks at
        # partition groups 32j of SPP128/R3P4). Stages interleave with later
        # blocks' fronts/backs.
        def tail_a(st, n0, n1):
            """seg preact + squash scalars; cols [n0,n1)."""
            st["sp4"] = pb.tile([128, 512], BF16, tag="sp4", name="sp4")
            nc.scalar.activation(st["sp4"][:, n0:n1],
                                 st["spp"][:, n0:n1],
                                 AF.Identity, bias=CB2V[:], scale=1.0)
            sp2 = pb.tile([128, 512], BF16, tag="sp2")
            nc.scalar.activation(sp2[:, n0:n1], st["spp"][:, n0:n1],
                                 AF.Square, bias=CB2V[:], scale=1.0)
            sq2 = pps.tile([4, 512], F32, tag="pps", name="sq2")
            st["sq2"] = sq2
            nc.tensor.matmul(sq2[0:4, n0:n1], INDSEG[:], sp2[0:112, n0:n1],
                             start=True, stop=True)
            t2 = pt.tile([4, 512], F32, tag="t2")
            st["t2"] = t2
            nc.scalar.activation(t2[:, n0:n1], sq2[0:4, n0:n1], AF.Sqrt,
                                 bias=EPS4[:], scale=1.0)
            u2 = pt.tile([4, 512], F32, tag="u2")
            nc.vector.tensor_scalar(out=u2[:, n0:n1], in0=sq2[0:4, n0:n1],
                                    scalar1=1.0, scalar2=None, op0=ADD)
            f2a = pt.tile([4, 512], F32, tag="f2a")
            st["f2a"] = f2a
            nc.vector.reciprocal_approx_fast(out=f2a[:, n0:n1], in_=u2[:, n0:n1])
            yt2 = pt.tile([4, 512], F32, tag="yt2")
            st["yt2"] = yt2
            nc.vector.tensor_mul(out=yt2[:, n0:n1], in0=st["yt"][:, n0:n1],
                                 in1=t2[:, n0:n1])

        def tail_b(st, n0, n1):
            """length out (sq2/(1+sq2), no sqrt), mask broadcast; [n0,n1)."""
            p0 = st["p0"]
            oseg = pt.tile([4, 512], BF16, tag="oseg")
            nc.vector.tensor_mul(out=oseg[:, n0:n1], in0=st["sq2"][0:4, n0:n1],
                                 in1=st["f2a"][:, n0:n1])
            nc.sync.dma_start(
                d["OSEG"][p0:p0 + 2048].rearrange("(p n) -> p n", p=4)[:, n0:n1],
                oseg[:, n0:n1])
            m1v = pt.tile([4, 512], BF16, tag="m1v")
            nc.vector.tensor_mul(out=m1v[:, n0:n1], in0=st["yt2"][:, n0:n1],
                                 in1=st["f2a"][:, n0:n1])
            bmp = pps.tile([128, 512], F32, tag="pps", name="bmp")
            nc.tensor.matmul(bmp[:, n0:n1], BCIND[:], m1v[:, n0:n1],
                             start=True, stop=True)
            nc.vector.tensor_mul(out=st["masked"][:, n0:n1],
                                 in0=st["sp4"][:, n0:n1],
                                 in1=bmp[:, n0:n1])

        def tail_r1(st, n0, n1):
            """recon stage 1: both block pairs' 64+64 hidden channels via two
            K=64 matmuls in disjoint PE row bands (concurrent)."""
            r1p = [None, None]
            for h in range(2):
                r1p[h] = pps.tile([128, 512], F32, tag="pps", name="r1p")
                nc.tensor.matmul(r1p[h][:, n0:n1], W1PAIR[h][:],
                                 st["masked"][64 * h:64 * h + 64, n0:n1],
                                 start=True, stop=True,
                                 tile_position=(64 * h, 0))
            for h in range(2):
                r1 = pa.tile([128, 512], BF16, tag="r1", name="r1", bufs=8)
                st[f"r1_{h}"] = r1
                eng = nc.scalar if h == 0 else nc.vector
                if h == 0:
                    nc.scalar.activation(r1[:, n0:n1], r1p[h][:, n0:n1],
                                         AF.Relu, bias=BR1[:], scale=1.0)
                else:
                    nc.vector.tensor_scalar(out=r1[:, n0:n1],
                                            in0=r1p[h][:, n0:n1],
                                            scalar1=BR1[:], scalar2=0.0,
                                            op0=ADD, op1=mybir.AluOpType.max)

        def tail_r2pair(st, jp, n0, n1):
            """recon stages 2+3 for block pair (2jp, 2jp+1): the two r2
            matmuls use disjoint K=64 row bands (concurrent); r3 accumulates
            into R3P4 at partition 32j via zero-padded WR3T columns."""
            r1 = st[f"r1_{jp}"]
            r2p = [None, None]
            for q in range(2):
                r2p[q] = pps.tile([128, 512], F32, tag="pps", name="r2p")
                nc.tensor.matmul(r2p[q][:, n0:n1], W2V[q][:],
                                 r1[64 * q:64 * q + 64, n0:n1],
                                 start=True, stop=True,
                                 tile_position=(64 * q, 0))
            r2 = [None, None]
            for q in range(2):
                r2[q] = pa.tile([128, 512], BF16, tag="r2", name="r2")
                if q == 0:
                    nc.scalar.activation(r2[q][:, n0:n1], r2p[q][:, n0:n1],
                                         AF.Relu, bias=BR2[:], scale=1.0)
                else:
                    nc.vector.tensor_scalar(out=r2[q][:, n0:n1],
                                            in0=r2p[q][:, n0:n1],
                                            scalar1=BR2[:], scalar2=0.0,
                                            op0=ADD, op1=mybir.AluOpType.max)
            for q in range(2):
                j = 2 * jp + q
                nc.tensor.matmul(st["r3p"][:, n0:n1], WR3T[j][:],
                                 r2[q][:, n0:n1],
                                 start=(j == 0), stop=(j == 3))

        def tail_sig(st, n0, n1, table=False):
            """sigmoid on R3P4 [97,512] (rows {0,32,64,96}); 4 output DMAs.
            table=True uses the ACT Sigmoid table in ONE op -- only legal
            after the last Sqrt use (the table swap evicts sqrt); the
            ~1.3us table load has no data deps and hides under the recon
            matmuls. Mid-kernel tails keep the DVE polynomial."""
            p0 = st["p0"]
            r3p = st["r3p"]
            if table:
                orect = pb.tile([97, 512], BF16, tag="orect", name="orect")
                nc.scalar.activation(orect[:, n0:n1], r3p[0:97, n0:n1],
                                     AF.Sigmoid, bias=BR3V[:], scale=1.0)
                nc.sync.dma_start(
                    d["OREC"][p0:p0 + 2048]
                    .rearrange("(p n) -> p n", p=4)[:, n0:n1],
                    orect[0:97:32, n0:n1])
                return
            # one scalar evac (psum -> sbuf), then the polynomial on the
            # otherwise-idle gpsimd engine: the vector queue stays free for
            # the last block's squash chain which runs concurrently.
            xv = pb.tile([97, 512], BF16, tag="xv")
            nc.scalar.activation(xv[:, n0:n1], r3p[0:97, n0:n1], AF.Identity,
                                 bias=BR3V[:], scale=1.0)
            x2 = pb.tile([97, 512], BF16, tag="x2")
            nc.gpsimd.tensor_mul(out=x2[:, n0:n1], in0=xv[:, n0:n1],
                                 in1=xv[:, n0:n1])
            x3 = pb.tile([97, 512], BF16, tag="x3")
            nc.gpsimd.tensor_mul(out=x3[:, n0:n1], in0=xv[:, n0:n1],
                                 in1=x2[:, n0:n1])
            v = pb.tile([97, 512], BF16, tag="v")
            nc.gpsimd.tensor_scalar(out=v[:, n0:n1], in0=x2[:, n0:n1],
                                    scalar1=1.0 / 480.0, scalar2=-1.0 / 48.0,
                                    op0=MULT, op1=ADD)
            r = pb.tile([97, 512], BF16, tag="r")
            nc.gpsimd.tensor_scalar(out=r[:, n0:n1], in0=xv[:, n0:n1],
                                    scalar1=0.25, scalar2=0.5,
                                    op0=MULT, op1=ADD)
            w = pb.tile([97, 512], BF16, tag="w")
            nc.gpsimd.tensor_mul(out=w[:, n0:n1], in0=x3[:, n0:n1],
                                 in1=v[:, n0:n1])
            orec = pb.tile([97, 512], BF16, tag="orec")
            nc.gpsimd.tensor_tensor(out=orec[:, n0:n1], in0=w[:, n0:n1],
                                    in1=r[:, n0:n1], op=ADD)
            for j, eng in enumerate((nc.sync, nc.gpsimd, nc.scalar, nc.sync)):
                eng.dma_start(
                    d["OREC"][p0 + 512 * j:p0 + 512 * j + 512]
                    .rearrange("(p n) -> p n", p=1)[:, n0:n1],
                    orec[32 * j:32 * j + 1, n0:n1])

        def sb_state(sb):
            st = {"p0": 2048 * sb, "sb": sb}
            st["masked"] = pa.tile([128, 512], BF16, tag="masked", name="masked")
            st["r3p"] = psp.tile([128, 512], F32, tag="r3p", name="r3p")
            st["yt"] = pt.tile([4, 512], F32, tag="yt", name="yt")
            nc.gpsimd.dma_start(
                st["yt"][:],
                d["YV"][st["p0"]:st["p0"] + 2048].rearrange("(p n) -> p n", p=4))
            return st

        # ---- schedule: block fronts run one block ahead of their backs so
        # the PE never stalls on the squash chain; sb0's tail interleaves
        # sb1's blocks; sb1's tail runs in two column halves.
        st0 = sb_state(0)
        st1 = sb_state(1)
        conv1_pair(0, 1)
        block_front(0)
        conv1_pair(2, 3, scalar_only=True)
        block_front(4)
        block_back(0, st0)
        block_front(8)
        block_back(4, st0)
        block_front(12)
        block_back(8, st0)
        block_front(16)
        block_back(12, st0)
        tail_a(st0, 0, 512)
        block_front(20)
        block_back(16, st1)
        tail_b(st0, 0, 512)
        block_front(24)
        block_back(20, st1)
        tail_r1(st0, 0, 512)
        block_last_m(28, 0)
        block_last_m(28, 1)
        block_back(24, st1)
        for jp in range(2):
            tail_r2pair(st0, jp, 0, 512)
        block_last_back(28, st1)
        tail_sig(st0, 0, 512)
        ha, hb = dict(st1), dict(st1)
        tail_a(ha, 0, 256)
        warm(2)
        tail_a(hb, 256, 512)
        warm(2)
        tail_b(ha, 0, 256)
        warm(2)
        tail_b(hb, 256, 512)
        warm(6)
        tail_r1(st1, 0, 512)
        warm(2)
        for jp in range(2):
            tail_r2pair(st1, jp, 0, 512)
            warm(2)
        tail_sig(st1, 0, 512, table=True)

    nc.compile()
    return nc


def _get_program():
    global _PROGRAM
    if _PROGRAM is None:
        _PROGRAM = _build_program()
    return _PROGRAM


def _host_prep(inputs):
    """Build per-core input maps from the full problem inputs."""
    x = np.asarray(inputs["x"], np.float32)
    y = np.asarray(inputs["y"], np.float32)
    W1 = np.asarray(inputs["W1"], np.float32)
    b1 = np.asarray(inputs["b1"], np.float32)
    Wp = np.asarray(inputs["Wp"], np.float32)
    bp = np.asarray(inputs["bp"], np.float32)
    cbp = np.asarray(inputs["cbp"], np.float32)
    Ws = np.asarray(inputs["Ws"], np.float32)
    bs = np.asarray(inputs["bs"], np.float32)
    cbs = np.asarray(inputs["cbs"], np.float32)
    Wr1 = np.asarray(inputs["Wr1"], np.float32)
    br1 = np.asarray(inputs["br1"], np.float32)
    Wr2 = np.asarray(inputs["Wr2"], np.float32)
    br2 = np.asarray(inputs["br2"], np.float32)
    Wr3 = np.asarray(inputs["Wr3"], np.float32)
    br3 = np.asarray(inputs["br3"], np.float32)

    W1r = W1.reshape(256, 25).T                      # [25 tap, 256 oc]
    W1T = np.concatenate([W1r, np.ones((1, 256), np.float32),
                          b1[None, :]], axis=0) * SA  # [27, 256], x SA
    W1T4 = np.zeros((128, 256), np.float32)
    for qt in range(4):
        W1T4[32 * qt:32 * qt + 27] = W1T
    WT8 = (np.ascontiguousarray(
        Wp.reshape(256, 2, 128, 25).transpose(1, 3, 2, 0)) * SW
    ).reshape(2, 25, 128, 256).transpose(2, 0, 1, 3)  # [128p, 2k, 25t, 256oc]
    WT8 = np.ascontiguousarray(WT8).astype(NP_F8)

    oc = np.arange(128)
    WsT = np.ascontiguousarray(Ws.reshape(16, 8).T[oc % 8])       # [128, 16]
    IND2 = (np.arange(128)[None, :] // 8 == np.arange(16)[:, None]).astype(np.float32)
    INDSQ = np.ascontiguousarray(IND2.T)
    cb1 = np.empty((128, 2), np.float32)
    for m in range(2):
        g = m * 128 + np.arange(128)
        cb1[:, m] = bp[g] / 32.0 + cbp[g // 8, g % 8, 0, 0]
    cb2 = (32.0 * bs + cbs[0, :, 0, 0]).astype(np.float32)

    Wr1T = Wr1.reshape(64, 16).T
    Wr2T = Wr2.reshape(128, 64).T

    packb = np.zeros((128, 1924), np.float32)
    packb[:, 0:16] = INDSQ                               # INDSQ m=0 -> parts 0-15
    packb[:, 96:112] = INDSQ                             # INDSQ m=1 -> parts 32-47
    for j in range(4):
        packb[:, 128 + 128 * j + 32 * j:128 + 128 * j + 32 * j + 16] = WsT
        packb[32 * j:32 * j + 16, 640 + j] = 1.0         # INDSEG
    packb[0:16, 644:772] = IND2                          # IND2a
    packb[32:48, 644:772] = IND2                         # IND2b
    for j in range(4):
        packb[j, 772 + 32 * j:772 + 32 * j + 16] = 1.0   # BCIND
        packb[:, 1156 + 128 * j + 32 * j] = Wr3.reshape(128)  # WR3T_j
    packb[0:64, 900:1028] = Wr2T                         # W2V[0]
    packb[64:128, 1028:1156] = Wr2T                      # W2V[1]
    for h in range(2):
        base = 1668 + 128 * h
        packb[64 * h:64 * h + 16, base:base + 64] = Wr1T
        packb[64 * h + 32:64 * h + 48, base + 64:base + 128] = Wr1T

    packf = np.zeros((128, 9), np.float32)
    packf[:, 0:2] = cb1
    for j in range(4):
        packf[32 * j:32 * j + 16, 2] = cb2               # CB2V
    packf[0:48, 3] = 1e-9                                # EPS48
    packf[0:4, 4] = 1e-9                                 # EPS4
    packf[0:64, 5] = br1
    packf[64:128, 5] = br1
    packf[:, 6] = br2
    packf[0:97:32, 7] = br3[0]                           # BR3V

    shared = {
        "W1T4": W1T4.astype(NP_BF16),
        "WT8": WT8,
        "PACKB": packb.astype(NP_BF16),
        "PACKF": packf,
    }

    in_maps = []
    for c in range(NCORES):
        b, j = divmod(c, NBLK)
        r0 = RB * j
        xpad = np.zeros((H + 8, W + 8), np.float32)
        xpad[4:4 + H, 4:4 + W] = x[b, 0]
        A = np.empty((27, RR, CW), np.float32)
        for dy in range(5):
            for dx in range(5):
                A[dy * 5 + dx] = xpad[r0 + dy:r0 + dy + RR, dx:dx + CW]
        # valid-mask row: -1e30 where the conv1 output position is padding
        rr = np.arange(RR)[:, None]
        cc = np.arange(CW)[None, :]
        valid = (r0 - 2 + rr >= 0) & (r0 - 2 + rr < H) & (cc >= 2) & (cc < 2 + W)
        A[25] = np.where(valid, 0.0, -1e30).astype(np.float32)
        A[26] = 1.0
        m = dict(shared)
        Af = A.reshape(27, AFLAT)
        A4 = np.zeros((128, QW), np.float32)
        for qt in range(4):
            A4[32 * qt:32 * qt + 27] = Af[:, QW * qt:QW * (qt + 1)]
        m["A4"] = A4.astype(NP_BF16)
        m["YV"] = np.ascontiguousarray(y[b, 0, r0:r0 + RB, :].reshape(NPX))
        in_maps.append(m)
    return in_maps


def _gather(results):
    out_seg = np.empty((B, 1, H, W), np.float32)
    out_rec = np.empty((B, 1, H, W), np.float32)
    for c in range(NCORES):
        b, j = divmod(c, NBLK)
        r0 = RB * j
        out_seg[b, 0, r0:r0 + RB, :] = \
            results[c]["OSEG"].astype(np.float32).reshape(RB, W)
        out_rec[b, 0, r0:r0 + RB, :] = \
            results[c]["OREC"].astype(np.float32).reshape(RB, W)
    return out_seg, out_rec


def kernel(**inputs):
    nc = _get_program()
    in_maps = _host_prep(inputs)
    res = run_bass_kernel_spmd(nc, in_maps, list(range(NCORES)))
    return _gather(res.results)



# revision 48
# speedup vs baseline: 1.0220x; 1.0220x over previous
"""CapsNet (nn_CapsNetBasic) forward pass as a Bass/Tile kernel on 8 TRN2 cores.

Sharding: 8 cores = 2 batch samples x 4 row-blocks of 32 output rows each.
Every core computes its 32x128-pixel slab end-to-end:
  conv1 (5x5, 1->256, bf16 im2col matmul with fused valid-mask/bias rows)
  primary caps conv (5x5, 256->256) in fp8e4m3 via DoubleRow matmuls: one
    instruction contracts both 128-channel halves per tap (25 matmuls/chain
    instead of 50) -- activations scaled x8, weights x64, rescaled in the
    preact activation (1/16384).
  per-capsule squash (partition-group reductions via 0/1 indicator matmuls)
  seg caps accumulated across 4 row-blocks into one PSUM tile at partition
    offsets {0,32,64,96} so the scalar tail (seg squash, length, masking,
    sigmoid) runs once per superblock on batched [4,512]/[128,512] tiles.
  recon 1x1 convs (16->64->128->1) per block, sigmoid via odd polynomial.
Superblock-0's tail matmuls are interleaved between superblock-1's primary
chains to keep the PE queue streaming; the final tail runs in two column
halves to shorten the drain. Routing softmaxes are constant for these shapes
(uniform 1/32 and singleton 1.0), so routing reduces to fixed reductions.

Scheduling notes (HW-measured):
- All tensor-path data is bf16/fp8: fp32 rhs runs the PE at 1/4 rate.
- The HAM clock gate starts the PE at 1.2 GHz and drops back whenever the
  PE idles ~a 3.4us window; a full-array dummy-matmul accumulation chain
  pre-warms it during the input DMA ramp, and short dummy chains woven
  into the final tail keep it warm across the serial squash spine.
- Matmuls emitted back-to-back whose lhsT tiles occupy disjoint PE row
  bands (tile_position) execute concurrently: used for conv1 quarter
  pairs, the bc broadcast m0/m1 pair, and the recon r1/r2 K=64 pairs.
- Engine queues are strictly in-order, so emission order must match
  data-ready order per engine; conv1's later-quarter evacuations go
  scalar-only so the vector queue never blocks a squash chain.
"""

import sys

sys.path.insert(0, "/opt/trn_rl_repo")

import numpy as np
import ml_dtypes
from contextlib import ExitStack

import concourse.bass as bass
import concourse.tile as tile
from concourse import mybir, bacc
from concourse.bass_utils import run_bass_kernel_spmd

F32 = mybir.dt.float32
F32R = mybir.dt.float32r
BF16 = mybir.dt.bfloat16
F8 = mybir.dt.float8e4
AF = mybir.ActivationFunctionType
DRMODE = mybir.MatmulPerfMode.DoubleRow
ADD = mybir.AluOpType.add
MULT = mybir.AluOpType.mult

B = 2
H = W = 128
RB = 32          # output rows per core
NBLK = 4         # row blocks per sample
NCORES = 8
RR = RB + 4      # conv1 buffer rows (halo 2 each side)
CW = W + 4       # padded width
AFLAT = RR * CW  # 4752
NPX = RB * W     # 4096 output pixels per core
QW = AFLAT // 4  # 1188 = 9 rows per conv1 quarter

SA = 8.0         # conv1-activation fp8 scale
SW = 64.0        # primary-conv weight fp8 scale
PSCALE = 1.0 / (32.0 * SA * SW)   # preact = psum*PSCALE + cb1

NP_BF16 = ml_dtypes.bfloat16
NP_F8 = ml_dtypes.float8_e4m3

INPUT_SHAPES = {
    "A4": (128, QW),              # bf16 im2col quarters
    "W1T4": (128, 256),           # bf16 conv1 weights (x SA)
    "WT8": (128, 2, 25, 256),     # fp8 primary conv weights (x SW)
    "YV": (NPX,),                 # f32 labels
    "PACKB": (128, 1924),         # bf16 matmul-constant pack
    "PACKF": (128, 9),            # f32 bias/eps pack
}

_PROGRAM = None


def _build_program():
    nc = bacc.Bacc("TRN2", target_bir_lowering=False, debug=False, num_devices=NCORES)

    d = {}
    dts = {"A4": BF16, "W1T4": BF16, "WT8": F8, "YV": F32,
           "PACKB": BF16, "PACKF": F32}
    for name, shape in INPUT_SHAPES.items():
        d[name] = nc.dram_tensor(name, list(shape), dts[name], kind="ExternalInput").ap()
    for name in ("OSEG", "OREC"):
        d[name] = nc.dram_tensor(name, [NPX], BF16, kind="ExternalOutput").ap()

    with tile.TileContext(nc) as tc, ExitStack() as ctx:
        pers = ctx.enter_context(tc.tile_pool(name="pers", bufs=1))
        pa = ctx.enter_context(tc.tile_pool(name="act", bufs=3))
        pb = ctx.enter_context(tc.tile_pool(name="bft", bufs=4))
        pt = ctx.enter_context(tc.tile_pool(name="tsm", bufs=4))
        ppc = ctx.enter_context(tc.tile_pool(name="ppc", bufs=2, space="PSUM"))
        pps = ctx.enter_context(tc.tile_pool(name="pps", bufs=3, space="PSUM"))

        # ---- persistent loads, ordered by earliest consumer. sync queue:
        # conv1's W1T4+A4 then the m0 DR weights in dy chunks (the m0 chains
        # consume taps incrementally); scalar queue: biases, the squash/seg
        # consts (PACKB cols <900), the m1 DR weights, then the tail consts.
        W1T4 = pers.tile([128, 256], BF16, tag="W1T4")
        nc.sync.dma_start(W1T4[:], d["W1T4"][:])
        A4 = pers.tile([128, QW], BF16, tag="A4")
        nc.sync.dma_start(A4[0:64, :], d["A4"][0:64, :])
        nc.sync.dma_start(A4[64:128, :], d["A4"][64:128, :])
        WT8 = pers.tile([128, 2, 25, 256], F8, tag="WT8")
        for dy0, dy1 in ((0, 2), (2, 4), (4, 5)):
            nc.sync.dma_start(WT8[:, 0, 5 * dy0:5 * dy1, :],
                              d["WT8"][:, 0, 5 * dy0:5 * dy1, :])
        PACKF = pers.tile([128, 9], F32, tag="PACKF")
        nc.scalar.dma_start(PACKF[:], d["PACKF"][:])
        PACKB = pers.tile([128, 1924], BF16, tag="PACKB")
        nc.scalar.dma_start(PACKB[:, 0:900], d["PACKB"][:, 0:900])
        for dy0, dy1 in ((0, 2), (2, 4), (4, 5)):
            nc.scalar.dma_start(WT8[:, 1, 5 * dy0:5 * dy1, :],
                                d["WT8"][:, 1, 5 * dy0:5 * dy1, :])
        nc.scalar.dma_start(PACKB[:, 900:1924], d["PACKB"][:, 900:1924])

        # ---- PE pre-warm. The HAM clock gate holds the PE at 1.2 GHz until
        # it has been busy ~3.4us; dummy matmuls on a zeroed scratch warm it
        # while the input DMAs are in flight, and later keep it warm across
        # the serial tail so the real tail matmuls run at 2.4 GHz.
        pdum = ctx.enter_context(tc.tile_pool(name="dum", bufs=1, space="PSUM"))
        DSC = pers.tile([128, 640], BF16, tag="DSC")
        nc.gpsimd.memset(DSC[:], 0.0)

        def warm(n, wN=512):
            # one full-array accumulation chain -> no inter-matmul semaphores
            # and full PE activity (HAM's busy detector needs both)
            dps = pdum.tile([128, 512], F32, tag="dum", name="warm")
            for i in range(n):
                nc.tensor.matmul(dps[:, :wN], DSC[:, 0:128],
                                 DSC[:, 128:128 + wN],
                                 start=(i == 0), stop=(i == n - 1))

        warm(15)

        # const views (all matmul outputs land at partition 0 or use
        # 32-aligned row bands; cross-partition placement happens via
        # zero-padded lhsT columns + psum accumulation)
        IND2a = PACKB[0:16, 644:772]     # capsule->atom broadcast, m=0 rows
        IND2b = PACKB[32:48, 644:772]    # copy for m=1 rows
        BCIND = PACKB[0:4, 772:900]      # block->group broadcast
        W2V = [PACKB[0:64, 900:1028], PACKB[64:128, 1028:1156]]
        WR3T = [PACKB[:, 1156 + 128 * j:1284 + 128 * j] for j in range(4)]
        W1PAIR = [PACKB[0:64, 1668:1796], PACKB[64:128, 1796:1924]]
        INDSQ = [PACKB[:, 64 * m:64 * m + 64] for m in range(2)]
        INDSQ16 = [PACKB[:, 0:16], PACKB[:, 96:112]]
        WsT4 = [PACKB[:, 128 + 128 * j:256 + 128 * j] for j in range(4)]
        INDSEG = PACKB[0:112, 640:644]   # group->block 0/1 reduction
        CB1 = PACKF[:, 0:2]
        CB2V = PACKF[:, 2:3]
        EPS48 = PACKF[0:48, 3:4]
        EPS4 = PACKF[0:4, 4:5]
        BR1 = PACKF[:, 5:6]              # rows 64-127 zero
        BR2 = PACKF[:, 6:7]
        BR3V = PACKF[0:97, 7:8]          # br3 at rows {0,32,64,96}

        # ---- conv1: 1->256 5x5 via host im2col (25 taps + valid-mask + bias
        # rows), 4 column-quarters on PE row bands {0,32,64,96}. relu out in
        # fp8 (x SA folded into W1T4), split across ACT (m=0) and Pool (m=1).
        C1 = pers.tile([128, 2, RR, CW], F8, tag="C1")
        C1F = [C1[:, m, :, :].rearrange("p r c -> p (r c)") for m in range(2)]
        _c1ctr = [0]

        def conv1_pair(qa, qb, scalar_only=False):
            """One m-half, one qoff chunk at a time for quarters qa and qb:
            the two matmuls sit in disjoint PE row bands (32qa / 32qb) so
            they execute concurrently. scalar_only keeps the vector engine
            free for squash chains when a pair runs between DR chains."""
            for m in range(2):
                for qoff in range(0, QW, 512):
                    n = min(512, QW - qoff)
                    ps2 = [None, None]
                    for i, qt in enumerate((qa, qb)):
                        # scalar_only pairs run between DR chains: keep their
                        # psums out of ppc so chains never wait on them via
                        # pool rotation
                        pool = ppc if (i == 0 and not scalar_only) else pps
                        ps2[i] = pool.tile([128, 512], F32,
                                           tag="ppc" if pool is ppc else "pps",
                                           name="c1ps")
                        nc.tensor.matmul(
                            ps2[i][:, :n],
                            W1T4[32 * qt:32 * qt + 27, m * 128:(m + 1) * 128],
                            A4[32 * qt:32 * qt + 27, qoff:qoff + n],
                            start=True, stop=True,
                            tile_position=(32 * qt, 0),
                        )
                    for i, qt in enumerate((qa, qb)):
                        _c1ctr[0] += 1
                        dst = C1F[m][:, QW * qt + qoff:QW * qt + qoff + n]
                        if scalar_only or _c1ctr[0] % 2 == 0:
                            nc.scalar.activation(dst, ps2[i][:, :n], AF.Relu,
                                                 bias=0.0, scale=1.0)
                        else:
                            nc.vector.tensor_scalar_max(dst, ps2[i][:, :n],
                                                        0.0)

        N = 512

        psp = ctx.enter_context(tc.tile_pool(name="psp", bufs=1, space="PSUM"))

        bst = {}

        def block_front(row0):
            """Primary DR conv + capsule square/reduce + squash scalar chain
            for one 4-row block. Both m halves' |s|^2 land in one SQ64 psum
            tile (m0 at partitions 0-15, m1 at 32-47) via zero-padded
            accumulating indicator matmuls."""
            st = {}
            bst[row0] = st
            P = [None, None]
            for m in range(2):
                ps = ppc.tile([128, 512], F32, tag="ppc")
                for t in range(25):
                    dy, dx = divmod(t, 5)
                    nc.tensor.matmul(
                        ps[:, :N],
                        WT8[:, :, t, m * 128:(m + 1) * 128],
                        C1[:, :, row0 + dy:row0 + dy + 4, dx:dx + 128],
                        start=(t == 0), stop=(t == 24),
                        perf_mode=DRMODE,
                    )
                P[m] = pb.tile([128, 512], BF16, tag="P", name="P")
                nc.scalar.activation(P[m][:, :N], ps[:, :N], AF.Identity,
                                     bias=CB1[:, m:m + 1], scale=PSCALE)
            st["P"] = P
            sq = pps.tile([64, 512], F32, tag="pps", name="sq")
            for m in range(2):
                S = pb.tile([128, 512], BF16, tag="S", name="S")
                nc.vector.tensor_mul(out=S[:, :N], in0=P[m][:, :N],
                                     in1=P[m][:, :N])
                nc.tensor.matmul(sq[0:64, :N], INDSQ[m][:], S[:, :N],
                                 start=(m == 0), stop=(m == 1))
            tq = pt.tile([48, 512], F32, tag="tq")
            nc.scalar.activation(tq[:, :N], sq[0:48, :N], AF.Sqrt,
                                 bias=EPS48[:], scale=1.0)
            u = pt.tile([48, 512], F32, tag="u")
            nc.vector.scalar_tensor_tensor(
                out=u[:, :N], in0=sq[0:48, :N], scalar=1.0, in1=tq[:, :N],
                op0=ADD, op1=MULT)
            rf0 = pt.tile([48, 512], F32, tag="rf0")
            nc.vector.reciprocal_approx_fast(out=rf0[:, :N], in_=u[:, :N])
            rf = pt.tile([48, 512], BF16, tag="rf")
            st["rf"] = rf
            nc.vector.tensor_mul(out=rf[:, :N], in0=sq[0:48, :N],
                                 in1=rf0[:, :N])

        def block_last_m(row0, m):
            """One m-half of the last block: DR chain + squash chain, so
            ready-early matmuls can be emitted between the two halves."""
            if row0 not in bst:
                rfb = pt.tile([48, 512], BF16, tag="rf", name="rf")
                bst[row0] = {"P": [None, None],
                             "rfs": [rfb[0:16, :], rfb[32:48, :]]}
            st = bst[row0]
            ps = ppc.tile([128, 512], F32, tag="ppc", name="ps")
            for t in range(25):
                dy, dx = divmod(t, 5)
                nc.tensor.matmul(
                    ps[:, :N],
                    WT8[:, :, t, m * 128:(m + 1) * 128],
                    C1[:, :, row0 + dy:row0 + dy + 4, dx:dx + 128],
                    start=(t == 0), stop=(t == 24),
                    perf_mode=DRMODE,
                )
            st["P"][m] = P = pb.tile([128, 512], BF16, tag="P", name="P")
            nc.scalar.activation(P[:, :N], ps[:, :N], AF.Identity,
                                 bias=CB1[:, m:m + 1], scale=PSCALE)
            S = pb.tile([128, 512], BF16, tag="S", name="S")
            nc.vector.tensor_mul(out=S[:, :N], in0=P[:, :N], in1=P[:, :N])
            sq = pps.tile([16, 512], F32, tag="pps", name="sq")
            nc.tensor.matmul(sq[0:16, :N], INDSQ16[m][:], S[:, :N],
                             start=True, stop=True)
            tq = pt.tile([16, 512], F32, tag="tq")
            nc.scalar.activation(tq[:, :N], sq[0:16, :N], AF.Sqrt,
                                 bias=EPS48[0:16], scale=1.0)
            u = pt.tile([16, 512], F32, tag="u")
            nc.vector.scalar_tensor_tensor(
                out=u[:, :N], in0=sq[0:16, :N], scalar=1.0,
                in1=tq[:, :N], op0=ADD, op1=MULT)
            rf0 = pt.tile([16, 512], F32, tag="rf0")
            nc.vector.reciprocal_approx_fast(out=rf0[:, :N], in_=u[:, :N])
            nc.vector.tensor_mul(out=st["rfs"][m][:, :N], in0=sq[0:16, :N],
                                 in1=rf0[:, :N])

        def block_last_back(row0, sbst):
            st = bst.pop(row0)
            j = (row0 % 16) // 4
            spp = sbst["spp"]
            P, rfs = st["P"], st["rfs"]
            bc = [None, None]
            for m in range(2):
                bc[m] = pps.tile([128, 512], F32, tag="pps", name="bc")
                nc.tensor.matmul(bc[m][:, :N], IND2a[:] if m == 0 else IND2b[:],
                                 rfs[m][:, :N], start=True, stop=True,
                                 tile_position=(32 * m, 0))
            pm = [None, None]
            for m in range(2):
                pm[m] = pb.tile([128, 512], BF16, tag="pm", name="pm")
                nc.vector.tensor_mul(out=pm[m][:, :N], in0=P[m][:, :N],
                                     in1=bc[m][:, :N])
            for m in range(2):
                nc.tensor.matmul(spp[:, :N], WsT4[j][:], pm[m][:, :N],
                                 start=False, stop=(m == 1))

        def block_back(row0, sbst):
            """Broadcast squash factors, apply, seg conv accumulating into
            the superblock's SPP128 (block j at partitions 32j via
            zero-padded WsT columns). Emitted one block behind the fronts
            so the PE never waits on the squash chain."""
            st = bst.pop(row0)
            j = (row0 % 16) // 4
            if j == 0:
                sbst["spp"] = psp.tile([128, 512], F32, tag="spp", name="spp")
            spp = sbst["spp"]
            # bc m0/m1 sit in disjoint PE row bands (0-15 / 32-47): emitted
            # back-to-back they execute concurrently in the array.
            bc = [None, None]
            for m in range(2):
                bc[m] = pps.tile([128, 512], F32, tag="pps", name="bc")
                nc.tensor.matmul(bc[m][:, :N], IND2a[:] if m == 0 else IND2b[:],
                                 st["rf"][32 * m:32 * m + 16, :N],
                                 start=True, stop=True,
                                 tile_position=(32 * m, 0))
            pm = [None, None]
            for m in range(2):
                pm[m] = pb.tile([128, 512], BF16, tag="pm", name="pm")
                nc.vector.tensor_mul(out=pm[m][:, :N], in0=st["P"][m][:, :N],
                                     in1=bc[m][:, :N])
            for m in range(2):
                nc.tensor.matmul(spp[:, :N], WsT4[j][:], pm[m][:, :N],
                                 start=(j == 0 and m == 0),
                                 stop=(j == 3 and m == 1))

        # ---- superblock tail (pixel range [p0, p0+2048), blocks at
        # partition groups 32j of SPP128/R3P4). Stages interleave with later
        # blocks' fronts/backs.
        def tail_a(st, n0, n1):
            """seg preact + squash scalars; cols [n0,n1)."""
            st["sp4"] = pb.tile([128, 512], BF16, tag="sp4", name="sp4")
            nc.scalar.activation(st["sp4"][:, n0:n1],
                                 st["spp"][:, n0:n1],
                                 AF.Identity, bias=CB2V[:], scale=1.0)
            sp2 = pb.tile([128, 512], BF16, tag="sp2")
            nc.scalar.activation(sp2[:, n0:n1], st["spp"][:, n0:n1],
                                 AF.Square, bias=CB2V[:], scale=1.0)
            sq2 = pps.tile([4, 512], F32, tag="pps", name="sq2")
            st["sq2"] = sq2
            nc.tensor.matmul(sq2[0:4, n0:n1], INDSEG[:], sp2[0:112, n0:n1],
                             start=True, stop=True)
            t2 = pt.tile([4, 512], F32, tag="t2")
            st["t2"] = t2
            nc.scalar.activation(t2[:, n0:n1], sq2[0:4, n0:n1], AF.Sqrt,
                                 bias=EPS4[:], scale=1.0)
            u2 = pt.tile([4, 512], F32, tag="u2")
            nc.vector.tensor_scalar(out=u2[:, n0:n1], in0=sq2[0:4, n0:n1],
                                    scalar1=1.0, scalar2=None, op0=ADD)
            f2a = pt.tile([4, 512], F32, tag="f2a")
            st["f2a"] = f2a
            nc.vector.reciprocal_approx_fast(out=f2a[:, n0:n1], in_=u2[:, n0:n1])
            yt2 = pt.tile([4, 512], F32, tag="yt2")
            st["yt2"] = yt2
            nc.gpsimd.tensor_mul(out=yt2[:, n0:n1], in0=st["yt"][:, n0:n1],
                                 in1=t2[:, n0:n1])

        def tail_b(st, n0, n1):
            """length out (sq2/(1+sq2), no sqrt), mask broadcast; [n0,n1)."""
            p0 = st["p0"]
            oseg = pt.tile([4, 512], BF16, tag="oseg")
            nc.vector.tensor_mul(out=oseg[:, n0:n1], in0=st["sq2"][0:4, n0:n1],
                                 in1=st["f2a"][:, n0:n1])
            nc.sync.dma_start(
                d["OSEG"][p0:p0 + 2048].rearrange("(p n) -> p n", p=4)[:, n0:n1],
                oseg[:, n0:n1])
            m1v = pt.tile([4, 512], BF16, tag="m1v")
            nc.gpsimd.tensor_mul(out=m1v[:, n0:n1], in0=st["yt2"][:, n0:n1],
                                 in1=st["f2a"][:, n0:n1])
            bmp = pps.tile([128, 512], F32, tag="pps", name="bmp")
            nc.tensor.matmul(bmp[:, n0:n1], BCIND[:], m1v[:, n0:n1],
                             start=True, stop=True)
            nc.vector.tensor_mul(out=st["masked"][:, n0:n1],
                                 in0=st["sp4"][:, n0:n1],
                                 in1=bmp[:, n0:n1])

        def tail_r1(st, n0, n1):
            """recon stage 1: both block pairs' 64+64 hidden channels via two
            K=64 matmuls in disjoint PE row bands (concurrent)."""
            r1p = [None, None]
            for h in range(2):
                r1p[h] = pps.tile([128, 512], F32, tag="pps", name="r1p")
                nc.tensor.matmul(r1p[h][:, n0:n1], W1PAIR[h][:],
                                 st["masked"][64 * h:64 * h + 64, n0:n1],
                                 start=True, stop=True,
                                 tile_position=(64 * h, 0))
            for h in range(2):
                r1 = pa.tile([128, 512], BF16, tag="r1", name="r1", bufs=8)
                st[f"r1_{h}"] = r1
                eng = nc.scalar if h == 0 else nc.vector
                if h == 0:
                    nc.scalar.activation(r1[:, n0:n1], r1p[h][:, n0:n1],
                                         AF.Relu, bias=BR1[:], scale=1.0)
                else:
                    nc.vector.tensor_scalar(out=r1[:, n0:n1],
                                            in0=r1p[h][:, n0:n1],
                                            scalar1=BR1[:], scalar2=0.0,
                                            op0=ADD, op1=mybir.AluOpType.max)

        def tail_r2pair(st, jp, n0, n1):
            """recon stages 2+3 for block pair (2jp, 2jp+1): the two r2
            matmuls use disjoint K=64 row bands (concurrent); r3 accumulates
            into R3P4 at partition 32j via zero-padded WR3T columns."""
            r1 = st[f"r1_{jp}"]
            r2p = [None, None]
            for q in range(2):
                r2p[q] = pps.tile([128, 512], F32, tag="pps", name="r2p")
                nc.tensor.matmul(r2p[q][:, n0:n1], W2V[q][:],
                                 r1[64 * q:64 * q + 64, n0:n1],
                                 start=True, stop=True,
                                 tile_position=(64 * q, 0))
            r2 = [None, None]
            for q in range(2):
                r2[q] = pa.tile([128, 512], BF16, tag="r2", name="r2")
                if q == 0:
                    nc.scalar.activation(r2[q][:, n0:n1], r2p[q][:, n0:n1],
                                         AF.Relu, bias=BR2[:], scale=1.0)
                else:
                    nc.vector.tensor_scalar(out=r2[q][:, n0:n1],
                                            in0=r2p[q][:, n0:n1],
                                            scalar1=BR2[:], scalar2=0.0,
                                            op0=ADD, op1=mybir.AluOpType.max)
            for q in range(2):
                j = 2 * jp + q
                nc.tensor.matmul(st["r3p"][:, n0:n1], WR3T[j][:],
                                 r2[q][:, n0:n1],
                                 start=(j == 0), stop=(j == 3))

        def tail_sig(st, n0, n1, table=False):
            """sigmoid on R3P4 [97,512] (rows {0,32,64,96}); 4 output DMAs.
            table=True uses the ACT Sigmoid table in ONE op -- only legal
            after the last Sqrt use (the table swap evicts sqrt); the
            ~1.3us table load has no data deps and hides under the recon
            matmuls. Mid-kernel tails keep the DVE polynomial."""
            p0 = st["p0"]
            r3p = st["r3p"]
            if table:
                orect = pb.tile([97, 512], BF16, tag="orect", name="orect")
                nc.scalar.activation(orect[:, n0:n1], r3p[0:97, n0:n1],
                                     AF.Sigmoid, bias=BR3V[:], scale=1.0)
                nc.sync.dma_start(
                    d["OREC"][p0:p0 + 2048]
                    .rearrange("(p n) -> p n", p=4)[:, n0:n1],
                    orect[0:97:32, n0:n1])
                return
            xv = pb.tile([97, 512], BF16, tag="xv")
            nc.vector.tensor_scalar(out=xv[:, n0:n1], in0=r3p[0:97, n0:n1],
                                    scalar1=BR3V[:], scalar2=None, op0=ADD)
            x2 = pb.tile([97, 512], BF16, tag="x2")
            nc.vector.tensor_mul(out=x2[:, n0:n1], in0=xv[:, n0:n1],
                                 in1=xv[:, n0:n1])
            x3 = pb.tile([97, 512], BF16, tag="x3")
            nc.vector.tensor_mul(out=x3[:, n0:n1], in0=xv[:, n0:n1],
                                 in1=x2[:, n0:n1])
            v = pb.tile([97, 512], BF16, tag="v")
            nc.vector.tensor_scalar(out=v[:, n0:n1], in0=x2[:, n0:n1],
                                    scalar1=1.0 / 480.0, scalar2=-1.0 / 48.0,
                                    op0=MULT, op1=ADD)
            r = pb.tile([97, 512], BF16, tag="r")
            nc.vector.tensor_scalar(out=r[:, n0:n1], in0=xv[:, n0:n1],
                                    scalar1=0.25, scalar2=0.5,
                                    op0=MULT, op1=ADD)
            w = pb.tile([97, 512], BF16, tag="w")
            nc.vector.tensor_mul(out=w[:, n0:n1], in0=x3[:, n0:n1],
                                 in1=v[:, n0:n1])
            orec = pb.tile([97, 512], BF16, tag="orec")
            nc.vector.tensor_tensor(out=orec[:, n0:n1], in0=w[:, n0:n1],
                                    in1=r[:, n0:n1], op=ADD)
            for j, eng in enumerate((nc.sync, nc.gpsimd, nc.scalar, nc.sync)):
                eng.dma_start(
                    d["OREC"][p0 + 512 * j:p0 + 512 * j + 512]
                    .rearrange("(p n) -> p n", p=1)[:, n0:n1],
                    orec[32 * j:32 * j + 1, n0:n1])

        def sb_state(sb):
            st = {"p0": 2048 * sb, "sb": sb}
            st["masked"] = pa.tile([128, 512], BF16, tag="masked", name="masked")
            st["r3p"] = psp.tile([128, 512], F32, tag="r3p", name="r3p")
            st["yt"] = pt.tile([4, 512], F32, tag="yt", name="yt")
            nc.gpsimd.dma_start(
                st["yt"][:],
                d["YV"][st["p0"]:st["p0"] + 2048].rearrange("(p n) -> p n", p=4))
            return st

        # ---- schedule: block fronts run one block ahead of their backs so
        # the PE never stalls on the squash chain; sb0's tail interleaves
        # sb1's blocks; sb1's tail runs in two column halves.
        st0 = sb_state(0)
        st1 = sb_state(1)
        conv1_pair(0, 1)
        block_front(0)
        conv1_pair(2, 3, scalar_only=True)
        block_front(4)
        block_back(0, st0)
        block_front(8)
        block_back(4, st0)
        block_front(12)
        block_back(8, st0)
        block_front(16)
        block_back(12, st0)
        tail_a(st0, 0, 512)
        block_front(20)
        block_back(16, st1)
        tail_b(st0, 0, 512)
        block_front(24)
        block_back(20, st1)
        tail_r1(st0, 0, 512)
        block_last_m(28, 0)
        block_last_m(28, 1)
        block_back(24, st1)
        for jp in range(2):
            tail_r2pair(st0, jp, 0, 512)
        block_last_back(28, st1)
        tail_sig(st0, 0, 512)
        ha, hb = dict(st1), dict(st1)
        tail_a(ha, 0, 256)
        warm(2)
        tail_a(hb, 256, 512)
        warm(2)
        tail_b(ha, 0, 256)
        warm(2)
        tail_b(hb, 256, 512)
        warm(6)
        tail_r1(st1, 0, 512)
        warm(2)
        for jp in range(2):
            tail_r2pair(st1, jp, 0, 512)
            warm(2)
        tail_sig(st1, 0, 512, table=True)

    nc.compile()
    return nc


def _get_program():
    global _PROGRAM
    if _PROGRAM is None:
        _PROGRAM = _build_program()
    return _PROGRAM


def _host_prep(inputs):
    """Build per-core input maps from the full problem inputs."""
    x = np.asarray(inputs["x"], np.float32)
    y = np.asarray(inputs["y"], np.float32)
    W1 = np.asarray(inputs["W1"], np.float32)
    b1 = np.asarray(inputs["b1"], np.float32)
    Wp = np.asarray(inputs["Wp"], np.float32)
    bp = np.asarray(inputs["bp"], np.float32)
    cbp = np.asarray(inputs["cbp"], np.float32)
    Ws = np.asarray(inputs["Ws"], np.float32)
    bs = np.asarray(inputs["bs"], np.float32)
    cbs = np.asarray(inputs["cbs"], np.float32)
    Wr1 = np.asarray(inputs["Wr1"], np.float32)
    br1 = np.asarray(inputs["br1"], np.float32)
    Wr2 = np.asarray(inputs["Wr2"], np.float32)
    br2 = np.asarray(inputs["br2"], np.float32)
    Wr3 = np.asarray(inputs["Wr3"], np.float32)
    br3 = np.asarray(inputs["br3"], np.float32)

    W1r = W1.reshape(256, 25).T                      # [25 tap, 256 oc]
    W1T = np.concatenate([W1r, np.ones((1, 256), np.float32),
                          b1[None, :]], axis=0) * SA  # [27, 256], x SA
    W1T4 = np.zeros((128, 256), np.float32)
    for qt in range(4):
        W1T4[32 * qt:32 * qt + 27] = W1T
    WT8 = (np.ascontiguousarray(
        Wp.reshape(256, 2, 128, 25).transpose(1, 3, 2, 0)) * SW
    ).reshape(2, 25, 128, 256).transpose(2, 0, 1, 3)  # [128p, 2k, 25t, 256oc]
    WT8 = np.ascontiguousarray(WT8).astype(NP_F8)

    oc = np.arange(128)
    WsT = np.ascontiguousarray(Ws.reshape(16, 8).T[oc % 8])       # [128, 16]
    IND2 = (np.arange(128)[None, :] // 8 == np.arange(16)[:, None]).astype(np.float32)
    INDSQ = np.ascontiguousarray(IND2.T)
    cb1 = np.empty((128, 2), np.float32)
    for m in range(2):
        g = m * 128 + np.arange(128)
        cb1[:, m] = bp[g] / 32.0 + cbp[g // 8, g % 8, 0, 0]
    cb2 = (32.0 * bs + cbs[0, :, 0, 0]).astype(np.float32)

    Wr1T = Wr1.reshape(64, 16).T
    Wr2T = Wr2.reshape(128, 64).T

    packb = np.zeros((128, 1924), np.float32)
    packb[:, 0:16] = INDSQ                               # INDSQ m=0 -> parts 0-15
    packb[:, 96:112] = INDSQ                             # INDSQ m=1 -> parts 32-47
    for j in range(4):
        packb[:, 128 + 128 * j + 32 * j:128 + 128 * j + 32 * j + 16] = WsT
        packb[32 * j:32 * j + 16, 640 + j] = 1.0         # INDSEG
    packb[0:16, 644:772] = IND2                          # IND2a
    packb[32:48, 644:772] = IND2                         # IND2b
    for j in range(4):
        packb[j, 772 + 32 * j:772 + 32 * j + 16] = 1.0   # BCIND
        packb[:, 1156 + 128 * j + 32 * j] = Wr3.reshape(128)  # WR3T_j
    packb[0:64, 900:1028] = Wr2T                         # W2V[0]
    packb[64:128, 1028:1156] = Wr2T                      # W2V[1]
    for h in range(2):
        base = 1668 + 128 * h
        packb[64 * h:64 * h + 16, base:base + 64] = Wr1T
        packb[64 * h + 32:64 * h + 48, base + 64:base + 128] = Wr1T

    packf = np.zeros((128, 9), np.float32)
    packf[:, 0:2] = cb1
    for j in range(4):
        packf[32 * j:32 * j + 16, 2] = cb2               # CB2V
    packf[0:48, 3] = 1e-9                                # EPS48
    packf[0:4, 4] = 1e-9                                 # EPS4
    packf[0:64, 5] = br1
    packf[64:128, 5] = br1
    packf[:, 6] = br2
    packf[0:97:32, 7] = br3[0]                           # BR3V

    shared = {
        "W1T4": W1T4.astype(NP_BF16),
        "WT8": WT8,
        "PACKB": packb.astype(NP_BF16),
        "PACKF": packf,
    }

    in_maps = []
    for c in range(NCORES):
        b, j = divmod(c, NBLK)
        r0 = RB * j
        xpad = np.zeros((H + 8, W + 8), np.float32)
        xpad[4:4 + H, 4:4 + W] = x[b, 0]
        A = np.empty((27, RR, CW), np.float32)
        for dy in range(5):
            for dx in range(5):
                A[dy * 5 + dx] = xpad[r0 + dy:r0 + dy + RR, dx:dx + CW]
        # valid-mask row: -1e30 where the conv1 output position is padding
        rr = np.arange(RR)[:, None]
        cc = np.arange(CW)[None, :]
        valid = (r0 - 2 + rr >= 0) & (r0 - 2 + rr < H) & (cc >= 2) & (cc < 2 + W)
        A[25] = np.where(valid, 0.0, -1e30).astype(np.float32)
        A[26] = 1.0
        m = dict(shared)
        Af = A.reshape(27, AFLAT)
        A4 = np.zeros((128, QW), np.float32)
        for qt in range(4):
            A4[32 * qt:32 * qt + 27] = Af[:, QW * qt:QW * (qt + 1)]
        m["A4"] = A4.astype(NP_BF16)
        m["YV"] = np.ascontiguousarray(y[b, 0, r0:r0 + RB, :].reshape(NPX))
        in_maps.append(m)
    return in_maps


def _gather(results):
    out_seg = np.empty((B, 1, H, W), np.float32)
    out_rec = np.empty((B, 1, H, W), np.float32)
    for c in range(NCORES):
        b, j = divmod(c, NBLK)
        r0 = RB * j
        out_seg[b, 0, r0:r0 + RB, :] = \
            results[c]["OSEG"].astype(np.float32).reshape(RB, W)
        out_rec[b, 0, r0:r0 + RB, :] = \
            results[c]["OREC"].astype(np.float32).reshape(RB, W)
    return out_seg, out_rec


def kernel(**inputs):
    nc = _get_program()
    in_maps = _host_prep(inputs)
    res = run_bass_kernel_spmd(nc, in_maps, list(range(NCORES)))
    return _gather(res.results)



# revision 50
# speedup vs baseline: 1.0295x; 1.0073x over previous
"""CapsNet (nn_CapsNetBasic) forward pass as a Bass/Tile kernel on 8 TRN2 cores.

Sharding: 8 cores = 2 batch samples x 4 row-blocks of 32 output rows each.
Every core computes its 32x128-pixel slab end-to-end:
  conv1 (5x5, 1->256, bf16 im2col matmul with fused valid-mask/bias rows)
  primary caps conv (5x5, 256->256) in fp8e4m3 via DoubleRow matmuls: one
    instruction contracts both 128-channel halves per tap (25 matmuls/chain
    instead of 50) -- activations scaled x8, weights x64, rescaled in the
    preact activation (1/16384).
  per-capsule squash (partition-group reductions via 0/1 indicator matmuls)
  seg caps accumulated across 4 row-blocks into one PSUM tile at partition
    offsets {0,32,64,96} so the scalar tail (seg squash, length, masking,
    sigmoid) runs once per superblock on batched [4,512]/[128,512] tiles.
  recon 1x1 convs (16->64->128->1) per block, sigmoid via odd polynomial.
Superblock-0's tail matmuls are interleaved between superblock-1's primary
chains to keep the PE queue streaming; the final tail runs in two column
halves to shorten the drain. Routing softmaxes are constant for these shapes
(uniform 1/32 and singleton 1.0), so routing reduces to fixed reductions.

Scheduling notes (HW-measured):
- All tensor-path data is bf16/fp8: fp32 rhs runs the PE at 1/4 rate.
- The HAM clock gate starts the PE at 1.2 GHz and drops back whenever the
  PE idles ~a 3.4us window; a full-array dummy-matmul accumulation chain
  pre-warms it during the input DMA ramp, and short dummy chains woven
  into the final tail keep it warm across the serial squash spine.
- Matmuls emitted back-to-back whose lhsT tiles occupy disjoint PE row
  bands (tile_position) execute concurrently: used for conv1 quarter
  pairs, the bc broadcast m0/m1 pair, and the recon r1/r2 K=64 pairs.
- Engine queues are strictly in-order, so emission order must match
  data-ready order per engine; conv1's later-quarter evacuations go
  scalar-only so the vector queue never blocks a squash chain.
"""

import sys

sys.path.insert(0, "/opt/trn_rl_repo")

import numpy as np
import ml_dtypes
from contextlib import ExitStack

import concourse.bass as bass
import concourse.tile as tile
from concourse import mybir, bacc
from concourse.bass_utils import run_bass_kernel_spmd

F32 = mybir.dt.float32
F32R = mybir.dt.float32r
BF16 = mybir.dt.bfloat16
F8 = mybir.dt.float8e4
AF = mybir.ActivationFunctionType
DRMODE = mybir.MatmulPerfMode.DoubleRow
ADD = mybir.AluOpType.add
MULT = mybir.AluOpType.mult

B = 2
H = W = 128
RB = 32          # output rows per core
NBLK = 4         # row blocks per sample
NCORES = 8
RR = RB + 4      # conv1 buffer rows (halo 2 each side)
CW = W + 4       # padded width
AFLAT = RR * CW  # 4752
NPX = RB * W     # 4096 output pixels per core
QW = AFLAT // 4  # 1188 = 9 rows per conv1 quarter

SA = 8.0         # conv1-activation fp8 scale
SW = 64.0        # primary-conv weight fp8 scale
PSCALE = 1.0 / (32.0 * SA * SW)   # preact = psum*PSCALE + cb1

NP_BF16 = ml_dtypes.bfloat16
NP_F8 = ml_dtypes.float8_e4m3

INPUT_SHAPES = {
    "A4": (128, QW),              # bf16 im2col quarters
    "W1T4": (128, 256),           # bf16 conv1 weights (x SA)
    "WT8": (128, 2, 25, 256),     # fp8 primary conv weights (x SW)
    "YV": (NPX,),                 # f32 labels
    "PACKB": (128, 1924),         # bf16 matmul-constant pack
    "PACKF": (128, 9),            # f32 bias/eps pack
}

_PROGRAM = None


def _build_program():
    nc = bacc.Bacc("TRN2", target_bir_lowering=False, debug=False, num_devices=NCORES)

    d = {}
    dts = {"A4": BF16, "W1T4": BF16, "WT8": F8, "YV": F32,
           "PACKB": BF16, "PACKF": F32}
    for name, shape in INPUT_SHAPES.items():
        d[name] = nc.dram_tensor(name, list(shape), dts[name], kind="ExternalInput").ap()
    for name in ("OSEG", "OREC"):
        d[name] = nc.dram_tensor(name, [NPX], BF16, kind="ExternalOutput").ap()

    with tile.TileContext(nc) as tc, ExitStack() as ctx:
        pers = ctx.enter_context(tc.tile_pool(name="pers", bufs=1))
        pa = ctx.enter_context(tc.tile_pool(name="act", bufs=3))
        pb = ctx.enter_context(tc.tile_pool(name="bft", bufs=4))
        pt = ctx.enter_context(tc.tile_pool(name="tsm", bufs=4))
        ppc = ctx.enter_context(tc.tile_pool(name="ppc", bufs=2, space="PSUM"))
        pps = ctx.enter_context(tc.tile_pool(name="pps", bufs=3, space="PSUM"))

        # ---- persistent loads, ordered by earliest consumer. sync queue:
        # conv1's W1T4+A4 then the m0 DR weights in dy chunks (the m0 chains
        # consume taps incrementally); scalar queue: biases, the squash/seg
        # consts (PACKB cols <900), the m1 DR weights, then the tail consts.
        W1T4 = pers.tile([128, 256], BF16, tag="W1T4")
        nc.sync.dma_start(W1T4[:], d["W1T4"][:])
        A4 = pers.tile([128, QW], BF16, tag="A4")
        nc.sync.dma_start(A4[0:64, :], d["A4"][0:64, :])
        nc.sync.dma_start(A4[64:128, :], d["A4"][64:128, :])
        WT8 = pers.tile([128, 2, 25, 256], F8, tag="WT8")
        for dy0, dy1 in ((0, 2), (2, 4), (4, 5)):
            nc.sync.dma_start(WT8[:, 0, 5 * dy0:5 * dy1, :],
                              d["WT8"][:, 0, 5 * dy0:5 * dy1, :])
        PACKF = pers.tile([128, 9], F32, tag="PACKF")
        nc.scalar.dma_start(PACKF[:], d["PACKF"][:])
        PACKB = pers.tile([128, 1924], BF16, tag="PACKB")
        nc.scalar.dma_start(PACKB[:, 0:900], d["PACKB"][:, 0:900])
        for dy0, dy1 in ((0, 2), (2, 4), (4, 5)):
            nc.scalar.dma_start(WT8[:, 1, 5 * dy0:5 * dy1, :],
                                d["WT8"][:, 1, 5 * dy0:5 * dy1, :])
        nc.scalar.dma_start(PACKB[:, 900:1924], d["PACKB"][:, 900:1924])

        # ---- PE pre-warm. The HAM clock gate holds the PE at 1.2 GHz until
        # it has been busy ~3.4us; dummy matmuls on a zeroed scratch warm it
        # while the input DMAs are in flight, and later keep it warm across
        # the serial tail so the real tail matmuls run at 2.4 GHz.
        pdum = ctx.enter_context(tc.tile_pool(name="dum", bufs=1, space="PSUM"))
        DSC = pers.tile([128, 640], BF16, tag="DSC")
        nc.gpsimd.memset(DSC[:], 0.0)

        def warm(n, wN=512):
            # one full-array accumulation chain -> no inter-matmul semaphores
            # and full PE activity (HAM's busy detector needs both)
            dps = pdum.tile([128, 512], F32, tag="dum", name="warm")
            for i in range(n):
                nc.tensor.matmul(dps[:, :wN], DSC[:, 0:128],
                                 DSC[:, 128:128 + wN],
                                 start=(i == 0), stop=(i == n - 1))

        warm(15)

        # const views (all matmul outputs land at partition 0 or use
        # 32-aligned row bands; cross-partition placement happens via
        # zero-padded lhsT columns + psum accumulation)
        IND2a = PACKB[0:16, 644:772]     # capsule->atom broadcast, m=0 rows
        IND2b = PACKB[32:48, 644:772]    # copy for m=1 rows
        BCIND = PACKB[0:4, 772:900]      # block->group broadcast
        W2V = [PACKB[0:64, 900:1028], PACKB[64:128, 1028:1156]]
        WR3T = [PACKB[:, 1156 + 128 * j:1284 + 128 * j] for j in range(4)]
        W1PAIR = [PACKB[0:64, 1668:1796], PACKB[64:128, 1796:1924]]
        INDSQ = [PACKB[:, 64 * m:64 * m + 64] for m in range(2)]
        INDSQ16 = [PACKB[:, 0:16], PACKB[:, 96:112]]
        WsT4 = [PACKB[:, 128 + 128 * j:256 + 128 * j] for j in range(4)]
        INDSEG = PACKB[0:112, 640:644]   # group->block 0/1 reduction
        CB1 = PACKF[:, 0:2]
        CB2V = PACKF[:, 2:3]
        EPS48 = PACKF[0:48, 3:4]
        EPS4 = PACKF[0:4, 4:5]
        BR1 = PACKF[:, 5:6]              # rows 64-127 zero
        BR2 = PACKF[:, 6:7]
        BR3V = PACKF[0:97, 7:8]          # br3 at rows {0,32,64,96}

        # ---- conv1: 1->256 5x5 via host im2col (25 taps + valid-mask + bias
        # rows), 4 column-quarters on PE row bands {0,32,64,96}. relu out in
        # fp8 (x SA folded into W1T4), split across ACT (m=0) and Pool (m=1).
        C1 = pers.tile([128, 2, RR, CW], F8, tag="C1")
        C1F = [C1[:, m, :, :].rearrange("p r c -> p (r c)") for m in range(2)]
        _c1ctr = [0]

        def conv1_pair(qa, qb, scalar_only=False):
            """One m-half, one qoff chunk at a time for quarters qa and qb:
            the two matmuls sit in disjoint PE row bands (32qa / 32qb) so
            they execute concurrently. scalar_only keeps the vector engine
            free for squash chains when a pair runs between DR chains."""
            for m in range(2):
                for qoff in range(0, QW, 512):
                    n = min(512, QW - qoff)
                    ps2 = [None, None]
                    for i, qt in enumerate((qa, qb)):
                        # scalar_only pairs run between DR chains: keep their
                        # psums out of ppc so chains never wait on them via
                        # pool rotation
                        pool = ppc if (i == 0 and not scalar_only) else pps
                        ps2[i] = pool.tile([128, 512], F32,
                                           tag="ppc" if pool is ppc else "pps",
                                           name="c1ps")
                        nc.tensor.matmul(
                            ps2[i][:, :n],
                            W1T4[32 * qt:32 * qt + 27, m * 128:(m + 1) * 128],
                            A4[32 * qt:32 * qt + 27, qoff:qoff + n],
                            start=True, stop=True,
                            tile_position=(32 * qt, 0),
                        )
                    for i, qt in enumerate((qa, qb)):
                        _c1ctr[0] += 1
                        dst = C1F[m][:, QW * qt + qoff:QW * qt + qoff + n]
                        if scalar_only or _c1ctr[0] % 2 == 0:
                            nc.scalar.activation(dst, ps2[i][:, :n], AF.Relu,
                                                 bias=0.0, scale=1.0)
                        else:
                            nc.vector.tensor_scalar_max(dst, ps2[i][:, :n],
                                                        0.0)

        N = 512

        psp = ctx.enter_context(tc.tile_pool(name="psp", bufs=1, space="PSUM"))

        bst = {}

        def block_front(row0):
            """Primary DR conv + capsule square/reduce + squash scalar chain
            for one 4-row block. Both m halves' |s|^2 land in one SQ64 psum
            tile (m0 at partitions 0-15, m1 at 32-47) via zero-padded
            accumulating indicator matmuls."""
            st = {}
            bst[row0] = st
            P = [None, None]
            for m in range(2):
                ps = ppc.tile([128, 512], F32, tag="ppc")
                for t in range(25):
                    dy, dx = divmod(t, 5)
                    nc.tensor.matmul(
                        ps[:, :N],
                        WT8[:, :, t, m * 128:(m + 1) * 128],
                        C1[:, :, row0 + dy:row0 + dy + 4, dx:dx + 128],
                        start=(t == 0), stop=(t == 24),
                        perf_mode=DRMODE,
                    )
                P[m] = pb.tile([128, 512], BF16, tag="P", name="P")
                nc.scalar.activation(P[m][:, :N], ps[:, :N], AF.Identity,
                                     bias=CB1[:, m:m + 1], scale=PSCALE)
            st["P"] = P
            sq = pps.tile([64, 512], F32, tag="pps", name="sq")
            for m in range(2):
                S = pb.tile([128, 512], BF16, tag="S", name="S")
                nc.vector.tensor_mul(out=S[:, :N], in0=P[m][:, :N],
                                     in1=P[m][:, :N])
                nc.tensor.matmul(sq[0:64, :N], INDSQ[m][:], S[:, :N],
                                 start=(m == 0), stop=(m == 1))
            tq = pt.tile([48, 512], F32, tag="tq")
            nc.scalar.activation(tq[:, :N], sq[0:48, :N], AF.Sqrt,
                                 bias=EPS48[:], scale=1.0)
            u = pt.tile([48, 512], F32, tag="u")
            nc.vector.scalar_tensor_tensor(
                out=u[:, :N], in0=sq[0:48, :N], scalar=1.0, in1=tq[:, :N],
                op0=ADD, op1=MULT)
            rf0 = pt.tile([48, 512], F32, tag="rf0")
            nc.vector.reciprocal_approx_fast(out=rf0[:, :N], in_=u[:, :N])
            rf = pt.tile([48, 512], BF16, tag="rf")
            st["rf"] = rf
            nc.vector.tensor_mul(out=rf[:, :N], in0=sq[0:48, :N],
                                 in1=rf0[:, :N])

        def block_last_m(row0, m):
            """One m-half of the last block: DR chain + squash chain, so
            ready-early matmuls can be emitted between the two halves."""
            if row0 not in bst:
                rfb = pt.tile([48, 512], BF16, tag="rf", name="rf")
                bst[row0] = {"P": [None, None],
                             "rfs": [rfb[0:16, :], rfb[32:48, :]]}
            st = bst[row0]
            ps = ppc.tile([128, 512], F32, tag="ppc", name="ps")
            for t in range(25):
                dy, dx = divmod(t, 5)
                nc.tensor.matmul(
                    ps[:, :N],
                    WT8[:, :, t, m * 128:(m + 1) * 128],
                    C1[:, :, row0 + dy:row0 + dy + 4, dx:dx + 128],
                    start=(t == 0), stop=(t == 24),
                    perf_mode=DRMODE,
                )
            st["P"][m] = P = pb.tile([128, 512], BF16, tag="P", name="P")
            nc.scalar.activation(P[:, :N], ps[:, :N], AF.Identity,
                                 bias=CB1[:, m:m + 1], scale=PSCALE)
            S = pb.tile([128, 512], BF16, tag="S", name="S")
            nc.vector.tensor_mul(out=S[:, :N], in0=P[:, :N], in1=P[:, :N])
            sq = pps.tile([16, 512], F32, tag="pps", name="sq")
            nc.tensor.matmul(sq[0:16, :N], INDSQ16[m][:], S[:, :N],
                             start=True, stop=True)
            tq = pt.tile([16, 512], F32, tag="tq")
            nc.scalar.activation(tq[:, :N], sq[0:16, :N], AF.Sqrt,
                                 bias=EPS48[0:16], scale=1.0)
            u = pt.tile([16, 512], F32, tag="u")
            nc.vector.scalar_tensor_tensor(
                out=u[:, :N], in0=sq[0:16, :N], scalar=1.0,
                in1=tq[:, :N], op0=ADD, op1=MULT)
            rf0 = pt.tile([16, 512], F32, tag="rf0")
            nc.vector.reciprocal_approx_fast(out=rf0[:, :N], in_=u[:, :N])
            nc.vector.tensor_mul(out=st["rfs"][m][:, :N], in0=sq[0:16, :N],
                                 in1=rf0[:, :N])

        def block_last_back(row0, sbst):
            st = bst.pop(row0)
            j = (row0 % 16) // 4
            spp = sbst["spp"]
            P, rfs = st["P"], st["rfs"]
            bc = [None, None]
            for m in range(2):
                bc[m] = pps.tile([128, 512], F32, tag="pps", name="bc")
                nc.tensor.matmul(bc[m][:, :N], IND2a[:] if m == 0 else IND2b[:],
                                 rfs[m][:, :N], start=True, stop=True,
                                 tile_position=(32 * m, 0))
            pm = [None, None]
            for m in range(2):
                pm[m] = pb.tile([128, 512], BF16, tag="pm", name="pm")
                nc.vector.tensor_mul(out=pm[m][:, :N], in0=P[m][:, :N],
                                     in1=bc[m][:, :N])
            for m in range(2):
                nc.tensor.matmul(spp[:, :N], WsT4[j][:], pm[m][:, :N],
                                 start=False, stop=(m == 1))

        def block_back(row0, sbst):
            """Broadcast squash factors, apply, seg conv accumulating into
            the superblock's SPP128 (block j at partitions 32j via
            zero-padded WsT columns). Emitted one block behind the fronts
            so the PE never waits on the squash chain."""
            st = bst.pop(row0)
            j = (row0 % 16) // 4
            if j == 0:
                sbst["spp"] = psp.tile([128, 512], F32, tag="spp", name="spp")
            spp = sbst["spp"]
            # bc m0/m1 sit in disjoint PE row bands (0-15 / 32-47): emitted
            # back-to-back they execute concurrently in the array.
            bc = [None, None]
            for m in range(2):
                bc[m] = pps.tile([128, 512], F32, tag="pps", name="bc")
                nc.tensor.matmul(bc[m][:, :N], IND2a[:] if m == 0 else IND2b[:],
                                 st["rf"][32 * m:32 * m + 16, :N],
                                 start=True, stop=True,
                                 tile_position=(32 * m, 0))
            pm = [None, None]
            for m in range(2):
                pm[m] = pb.tile([128, 512], BF16, tag="pm", name="pm")
                nc.vector.tensor_mul(out=pm[m][:, :N], in0=st["P"][m][:, :N],
                                     in1=bc[m][:, :N])
            for m in range(2):
                nc.tensor.matmul(spp[:, :N], WsT4[j][:], pm[m][:, :N],
                                 start=(j == 0 and m == 0),
                                 stop=(j == 3 and m == 1))

        # ---- superblock tail (pixel range [p0, p0+2048), blocks at
        # partition groups 32j of SPP128/R3P4). Stages interleave with later
        # blocks' fronts/backs.
        def tail_a(st, n0, n1):
            """seg preact + squash scalars; cols [n0,n1)."""
            st["sp4"] = pb.tile([128, 512], BF16, tag="sp4", name="sp4")
            nc.scalar.activation(st["sp4"][:, n0:n1],
                                 st["spp"][:, n0:n1],
                                 AF.Identity, bias=CB2V[:], scale=1.0)
            sp2 = pb.tile([128, 512], BF16, tag="sp2")
            nc.scalar.activation(sp2[:, n0:n1], st["spp"][:, n0:n1],
                                 AF.Square, bias=CB2V[:], scale=1.0)
            sq2 = pps.tile([4, 512], F32, tag="pps", name="sq2")
            st["sq2"] = sq2
            nc.tensor.matmul(sq2[0:4, n0:n1], INDSEG[:], sp2[0:112, n0:n1],
                             start=True, stop=True)
            t2 = pt.tile([4, 512], F32, tag="t2")
            st["t2"] = t2
            nc.scalar.activation(t2[:, n0:n1], sq2[0:4, n0:n1], AF.Sqrt,
                                 bias=EPS4[:], scale=1.0)
            u2 = pt.tile([4, 512], F32, tag="u2")
            nc.vector.tensor_scalar(out=u2[:, n0:n1], in0=sq2[0:4, n0:n1],
                                    scalar1=1.0, scalar2=None, op0=ADD)
            f2a = pt.tile([4, 512], F32, tag="f2a")
            st["f2a"] = f2a
            nc.vector.reciprocal_approx_fast(out=f2a[:, n0:n1], in_=u2[:, n0:n1])
            yt2 = pt.tile([4, 512], F32, tag="yt2")
            st["yt2"] = yt2
            nc.gpsimd.tensor_mul(out=yt2[:, n0:n1], in0=st["yt"][:, n0:n1],
                                 in1=t2[:, n0:n1])

        def tail_b(st, n0, n1):
            """length out (sq2/(1+sq2), no sqrt), mask broadcast; [n0,n1)."""
            p0 = st["p0"]
            oseg = pt.tile([4, 512], BF16, tag="oseg")
            nc.vector.tensor_mul(out=oseg[:, n0:n1], in0=st["sq2"][0:4, n0:n1],
                                 in1=st["f2a"][:, n0:n1])
            nc.sync.dma_start(
                d["OSEG"][p0:p0 + 2048].rearrange("(p n) -> p n", p=4)[:, n0:n1],
                oseg[:, n0:n1])
            m1v = pt.tile([4, 512], BF16, tag="m1v")
            nc.gpsimd.tensor_mul(out=m1v[:, n0:n1], in0=st["yt2"][:, n0:n1],
                                 in1=st["f2a"][:, n0:n1])
            bmp = pps.tile([128, 512], F32, tag="pps", name="bmp")
            nc.tensor.matmul(bmp[:, n0:n1], BCIND[:], m1v[:, n0:n1],
                             start=True, stop=True)
            nc.vector.tensor_mul(out=st["masked"][:, n0:n1],
                                 in0=st["sp4"][:, n0:n1],
                                 in1=bmp[:, n0:n1])

        def tail_r1(st, n0, n1):
            """recon stage 1: both block pairs' 64+64 hidden channels via two
            K=64 matmuls in disjoint PE row bands (concurrent)."""
            r1p = [None, None]
            for h in range(2):
                r1p[h] = pps.tile([128, 512], F32, tag="pps", name="r1p")
                nc.tensor.matmul(r1p[h][:, n0:n1], W1PAIR[h][:],
                                 st["masked"][64 * h:64 * h + 64, n0:n1],
                                 start=True, stop=True,
                                 tile_position=(64 * h, 0))
            for h in range(2):
                r1 = pa.tile([128, 512], BF16, tag="r1", name="r1", bufs=8)
                st[f"r1_{h}"] = r1
                eng = nc.scalar if h == 0 else nc.vector
                if h == 0:
                    nc.scalar.activation(r1[:, n0:n1], r1p[h][:, n0:n1],
                                         AF.Relu, bias=BR1[:], scale=1.0)
                else:
                    nc.vector.tensor_scalar(out=r1[:, n0:n1],
                                            in0=r1p[h][:, n0:n1],
                                            scalar1=BR1[:], scalar2=0.0,
                                            op0=ADD, op1=mybir.AluOpType.max)

        def tail_r2pair(st, jp, n0, n1):
            """recon stages 2+3 for block pair (2jp, 2jp+1): the two r2
            matmuls use disjoint K=64 row bands (concurrent); r3 accumulates
            into R3P4 at partition 32j via zero-padded WR3T columns."""
            r1 = st[f"r1_{jp}"]
            r2p = [None, None]
            for q in range(2):
                r2p[q] = pps.tile([128, 512], F32, tag="pps", name="r2p")
                nc.tensor.matmul(r2p[q][:, n0:n1], W2V[q][:],
                                 r1[64 * q:64 * q + 64, n0:n1],
                                 start=True, stop=True,
                                 tile_position=(64 * q, 0))
            r2 = [None, None]
            for q in range(2):
                r2[q] = pa.tile([128, 512], BF16, tag="r2", name="r2")
                if q == 0:
                    nc.scalar.activation(r2[q][:, n0:n1], r2p[q][:, n0:n1],
                                         AF.Relu, bias=BR2[:], scale=1.0)
                else:
                    nc.vector.tensor_scalar(out=r2[q][:, n0:n1],
                                            in0=r2p[q][:, n0:n1],
                                            scalar1=BR2[:], scalar2=0.0,
                                            op0=ADD, op1=mybir.AluOpType.max)
            for q in range(2):
                j = 2 * jp + q
                nc.tensor.matmul(st["r3p"][:, n0:n1], WR3T[j][:],
                                 r2[q][:, n0:n1],
                                 start=(j == 0), stop=(j == 3))

        def tail_sig(st, n0, n1, table=False):
            """sigmoid on R3P4 [97,512] (rows {0,32,64,96}); 4 output DMAs.
            table=True uses the ACT Sigmoid table in ONE op -- only legal
            after the last Sqrt use (the table swap evicts sqrt); the
            ~1.3us table load has no data deps and hides under the recon
            matmuls. Mid-kernel tails keep the DVE polynomial."""
            p0 = st["p0"]
            r3p = st["r3p"]
            if table:
                orect = pb.tile([97, 512], BF16, tag="orect", name="orect")
                nc.scalar.activation(orect[:, n0:n1], r3p[0:97, n0:n1],
                                     AF.Sigmoid, bias=BR3V[:], scale=1.0)
                nc.sync.dma_start(
                    d["OREC"][p0:p0 + 2048]
                    .rearrange("(p n) -> p n", p=4)[:, n0:n1],
                    orect[0:97:32, n0:n1])
                return
            xv = pb.tile([97, 512], BF16, tag="xv")
            nc.vector.tensor_scalar(out=xv[:, n0:n1], in0=r3p[0:97, n0:n1],
                                    scalar1=BR3V[:], scalar2=None, op0=ADD)
            x2 = pb.tile([97, 512], BF16, tag="x2")
            nc.vector.tensor_mul(out=x2[:, n0:n1], in0=xv[:, n0:n1],
                                 in1=xv[:, n0:n1])
            x3 = pb.tile([97, 512], BF16, tag="x3")
            nc.vector.tensor_mul(out=x3[:, n0:n1], in0=xv[:, n0:n1],
                                 in1=x2[:, n0:n1])
            v = pb.tile([97, 512], BF16, tag="v")
            nc.vector.tensor_scalar(out=v[:, n0:n1], in0=x2[:, n0:n1],
                                    scalar1=1.0 / 480.0, scalar2=-1.0 / 48.0,
                                    op0=MULT, op1=ADD)
            r = pb.tile([97, 512], BF16, tag="r")
            nc.vector.tensor_scalar(out=r[:, n0:n1], in0=xv[:, n0:n1],
                                    scalar1=0.25, scalar2=0.5,
                                    op0=MULT, op1=ADD)
            w = pb.tile([97, 512], BF16, tag="w")
            nc.vector.tensor_mul(out=w[:, n0:n1], in0=x3[:, n0:n1],
                                 in1=v[:, n0:n1])
            orec = pb.tile([97, 512], BF16, tag="orec")
            nc.vector.tensor_tensor(out=orec[:, n0:n1], in0=w[:, n0:n1],
                                    in1=r[:, n0:n1], op=ADD)
            for j, eng in enumerate((nc.sync, nc.gpsimd, nc.scalar, nc.sync)):
                eng.dma_start(
                    d["OREC"][p0 + 512 * j:p0 + 512 * j + 512]
                    .rearrange("(p n) -> p n", p=1)[:, n0:n1],
                    orec[32 * j:32 * j + 1, n0:n1])

        def sb_state(sb):
            st = {"p0": 2048 * sb, "sb": sb}
            st["masked"] = pa.tile([128, 512], BF16, tag="masked", name="masked")
            st["r3p"] = psp.tile([128, 512], F32, tag="r3p", name="r3p")
            st["yt"] = pt.tile([4, 512], F32, tag="yt", name="yt")
            nc.gpsimd.dma_start(
                st["yt"][:],
                d["YV"][st["p0"]:st["p0"] + 2048].rearrange("(p n) -> p n", p=4))
            return st

        # ---- schedule: block fronts run one block ahead of their backs so
        # the PE never stalls on the squash chain; sb0's tail interleaves
        # sb1's blocks; sb1's tail runs in two column halves.
        st0 = sb_state(0)
        st1 = sb_state(1)
        conv1_pair(0, 1)
        block_front(0)
        conv1_pair(2, 3, scalar_only=True)
        block_front(4)
        block_back(0, st0)
        block_front(8)
        block_back(4, st0)
        block_front(12)
        block_back(8, st0)
        block_front(16)
        block_back(12, st0)
        tail_a(st0, 0, 512)
        block_front(20)
        block_back(16, st1)
        tail_b(st0, 0, 512)
        block_front(24)
        block_back(20, st1)
        tail_r1(st0, 0, 512)
        block_last_m(28, 0)
        block_last_m(28, 1)
        block_back(24, st1)
        for jp in range(2):
            tail_r2pair(st0, jp, 0, 512)
        block_last_back(28, st1)
        tail_sig(st0, 0, 512)
        ha, hb = dict(st1), dict(st1)
        tail_a(ha, 0, 256)
        warm(2)
        tail_a(hb, 256, 512)
        warm(2)
        tail_b(ha, 0, 256)
        warm(2)
        tail_b(hb, 256, 512)
        warm(6)
        tail_r1(st1, 0, 512)
        warm(2)
        for jp in range(2):
            tail_r2pair(st1, jp, 0, 512)
            warm(2)
        tail_sig(st1, 0, 512, table=True)

    nc.compile()
    return nc


def _get_program():
    global _PROGRAM
    if _PROGRAM is None:
        _PROGRAM = _build_program()
    return _PROGRAM


def _host_prep(inputs):
    """Build per-core input maps from the full problem inputs."""
    x = np.asarray(inputs["x"], np.float32)
    y = np.asarray(inputs["y"], np.float32)
    W1 = np.asarray(inputs["W1"], np.float32)
    b1 = np.asarray(inputs["b1"], np.float32)
    Wp = np.asarray(inputs["Wp"], np.float32)
    bp = np.asarray(inputs["bp"], np.float32)
    cbp = np.asarray(inputs["cbp"], np.float32)
    Ws = np.asarray(inputs["Ws"], np.float32)
    bs = np.asarray(inputs["bs"], np.float32)
    cbs = np.asarray(inputs["cbs"], np.float32)
    Wr1 = np.asarray(inputs["Wr1"], np.float32)
    br1 = np.asarray(inputs["br1"], np.float32)
    Wr2 = np.asarray(inputs["Wr2"], np.float32)
    br2 = np.asarray(inputs["br2"], np.float32)
    Wr3 = np.asarray(inputs["Wr3"], np.float32)
    br3 = np.asarray(inputs["br3"], np.float32)

    W1r = W1.reshape(256, 25).T                      # [25 tap, 256 oc]
    W1T = np.concatenate([W1r, np.ones((1, 256), np.float32),
                          b1[None, :]], axis=0) * SA  # [27, 256], x SA
    W1T4 = np.zeros((128, 256), np.float32)
    for qt in range(4):
        W1T4[32 * qt:32 * qt + 27] = W1T
    WT8 = (np.ascontiguousarray(
        Wp.reshape(256, 2, 128, 25).transpose(1, 3, 2, 0)) * SW
    ).reshape(2, 25, 128, 256).transpose(2, 0, 1, 3)  # [128p, 2k, 25t, 256oc]
    WT8 = np.ascontiguousarray(WT8).astype(NP_F8)

    oc = np.arange(128)
    WsT = np.ascontiguousarray(Ws.reshape(16, 8).T[oc % 8])       # [128, 16]
    IND2 = (np.arange(128)[None, :] // 8 == np.arange(16)[:, None]).astype(np.float32)
    INDSQ = np.ascontiguousarray(IND2.T)
    cb1 = np.empty((128, 2), np.float32)
    for m in range(2):
        g = m * 128 + np.arange(128)
        cb1[:, m] = bp[g] / 32.0 + cbp[g // 8, g % 8, 0, 0]
    cb2 = (32.0 * bs + cbs[0, :, 0, 0]).astype(np.float32)

    Wr1T = Wr1.reshape(64, 16).T
    Wr2T = Wr2.reshape(128, 64).T

    packb = np.zeros((128, 1924), np.float32)
    packb[:, 0:16] = INDSQ                               # INDSQ m=0 -> parts 0-15
    packb[:, 96:112] = INDSQ                             # INDSQ m=1 -> parts 32-47
    for j in range(4):
        packb[:, 128 + 128 * j + 32 * j:128 + 128 * j + 32 * j + 16] = WsT
        packb[32 * j:32 * j + 16, 640 + j] = 1.0         # INDSEG
    packb[0:16, 644:772] = IND2                          # IND2a
    packb[32:48, 644:772] = IND2                         # IND2b
    for j in range(4):
        packb[j, 772 + 32 * j:772 + 32 * j + 16] = 1.0   # BCIND
        packb[:, 1156 + 128 * j + 32 * j] = Wr3.reshape(128)  # WR3T_j
    packb[0:64, 900:1028] = Wr2T                         # W2V[0]
    packb[64:128, 1028:1156] = Wr2T                      # W2V[1]
    for h in range(2):
        base = 1668 + 128 * h
        packb[64 * h:64 * h + 16, base:base + 64] = Wr1T
        packb[64 * h + 32:64 * h + 48, base + 64:base + 128] = Wr1T

    packf = np.zeros((128, 9), np.float32)
    packf[:, 0:2] = cb1
    for j in range(4):
        packf[32 * j:32 * j + 16, 2] = cb2               # CB2V
    packf[0:48, 3] = 1e-9                                # EPS48
    packf[0:4, 4] = 1e-9                                 # EPS4
    packf[0:64, 5] = br1
    packf[64:128, 5] = br1
    packf[:, 6] = br2
    packf[0:97:32, 7] = br3[0]                           # BR3V

    shared = {
        "W1T4": W1T4.astype(NP_BF16),
        "WT8": WT8,
        "PACKB": packb.astype(NP_BF16),
        "PACKF": packf,
    }

    in_maps = []
    for c in range(NCORES):
        b, j = divmod(c, NBLK)
        r0 = RB * j
        xpad = np.zeros((H + 8, W + 8), np.float32)
        xpad[4:4 + H, 4:4 + W] = x[b, 0]
        A = np.empty((27, RR, CW), np.float32)
        for dy in range(5):
            for dx in range(5):
                A[dy * 5 + dx] = xpad[r0 + dy:r0 + dy + RR, dx:dx + CW]
        # valid-mask row: -1e30 where the conv1 output position is padding
        rr = np.arange(RR)[:, None]
        cc = np.arange(CW)[None, :]
        valid = (r0 - 2 + rr >= 0) & (r0 - 2 + rr < H) & (cc >= 2) & (cc < 2 + W)
        A[25] = np.where(valid, 0.0, -1e30).astype(np.float32)
        A[26] = 1.0
        m = dict(shared)
        Af = A.reshape(27, AFLAT)
        A4 = np.zeros((128, QW), np.float32)
        for qt in range(4):
            A4[32 * qt:32 * qt + 27] = Af[:, QW * qt:QW * (qt + 1)]
        m["A4"] = A4.astype(NP_BF16)
        m["YV"] = np.ascontiguousarray(y[b, 0, r0:r0 + RB, :].reshape(NPX))
        in_maps.append(m)
    return in_maps


def _gather(results):
    out_seg = np.empty((B, 1, H, W), np.float32)
    out_rec = np.empty((B, 1, H, W), np.float32)
    for c in range(NCORES):
        b, j = divmod(c, NBLK)
        r0 = RB * j
        out_seg[b, 0, r0:r0 + RB, :] = \
            results[c]["OSEG"].astype(np.float32).reshape(RB, W)
        out_rec[b, 0, r0:r0 + RB, :] = \
            results[c]["OREC"].astype(np.float32).reshape(RB, W)
    return out_seg, out_rec


def kernel(**inputs):
    nc = _get_program()
    in_maps = _host_prep(inputs)
    res = run_bass_kernel_spmd(nc, in_maps, list(range(NCORES)))
    return _gather(res.results)

